# revision 24
# baseline (speedup 1.0000x reference)
"""Trainium2 Bass kernel for nn_DCGAN_G (DCGAN generator + 69-neuron spiking scan).

Strategy (8 NeuronCores, SPMD):
  A. W_in matvec (12800x2048) row-sharded 8x -> AllGather h1 (12800).
  B. DCGAN conv stack replicated on every core (tiny: ~3 GMAC).
  C. W_d2 matvec (4761x6400) row-sharded 8x -> AllGather w (69x69).
  D. 99800-step spiking recurrence in "primed" coordinates
     s'_t = (-1)^t s_t (tanh odd => u'_t = tanh(s'_t @ w)):
       serial phase (3048 steps), y'-space 2-op steps:
         y'_{t+1} = y'_t - u'_t @ w  (PSUM-accumulating matmul + tanh);
         trajectory emitted t-major per 127-step group (PE transpose +
         prefix matmul + sign multiply + direct DMA to the output).
       pipelined blocked-Picard phase: 127-step blocks, M in {2,3}
       iterations of {S' = prefix(U', s'0) via matmul with a triangular
       constant; Y' = S'@w; U' = tanh(Y')}.  Software-pipelined across
       blocks with "last-iteration-true" (LIT) semantics: iterations
       1..M-1 use a previewed start state (prefix-sum of the previous
       block's U^{(M-1)} via a ones-column matmul), only the final
       iteration waits for the previous block's converged end state.
       Critical path per block = 6 engine ops instead of 4M+2.
     Outputs are produced t-major per block (finish matmul with the
     triangular constant -> sign multiply -> batched DMA straight into
     the (T,69) output), eliminating the i-major trajectory round-trip
     and final transpose pass entirely.
     Host-validated (exact op-order mirror) vs jax ref: rel ~ 1.0e-3.
"""
import numpy as np

import bass_rust
import concourse.bass as bass
import concourse.mybir as mybir
from concourse.bass_utils import run_bass_kernel_spmd
from concourse.tile import TileContext
from concourse.vector_clock import ScopedClock

f32 = mybir.dt.float32
AF = mybir.ActivationFunctionType
OP = mybir.AluOpType
AX = mybir.AxisListType

T_FULL = 99800
N = 69
NCORES = 8
EPS = 1e-5
MROWS_A = 1600        # W_in rows per core
MROWS_C = 596         # W_d2 rows per core (8*596=4768 >= 4761)
KB = 127              # picard block length / serial group length
SER_G = 24            # serial groups (24*127 = 3048 serial steps)
N_M3 = 54             # leading picard blocks run M=3; the rest M=2
TAIL = 105            # tail block length (3048 + 761*127 + 105 == 99800)
DMA_GRP = 8           # picard blocks per output DMA


# ---------------------------------------------------------------------------
# walrus workaround: CTRL-type instructions accept at most 1 sem wait, but the
# TileContext tail drain gets one wait per active proc. Split across drains.
def _patched_drain_and_barrier(self, tick_clock, wait_clock):
    drain_inst = self.nc.sync.drain()
    wait_clock.add_sem_waits(
        drain_inst.ins, ScopedClock({None: tick_clock.global_clock})
    )
    si = drain_inst.ins.sync_info
    waits = list(si.on_wait) if si is not None else []
    if len(waits) > 1:
        drain_inst.ins.sync_info = bass_rust.SyncInfo(
            on_wait=waits[:1], on_update=list(si.on_update)
        )
        for i in range(1, len(waits)):
            extra = self.nc.sync.drain()
            extra.ins.sync_info = bass_rust.SyncInfo(
                on_wait=waits[i : i + 1], on_update=[]
            )
    self.nc.all_engine_barrier()
    assert self.sems is not None
    popped = self.nc._tile_sem_poison_stack.pop()
    assert popped is self._sem_poison
    self.nc.clear_and_free_semaphores(list(self.sems.allocated().values()))
    self.nc.all_engine_barrier()


TileContext._drain_and_barrier = _patched_drain_and_barrier
# ---------------------------------------------------------------------------


def _split_excess_waits(nc, max_waits=1):
    """This walrus build accepts at most one sem wait per instruction; move
    excess waits onto single-wait NOPs inserted just before the owner."""
    n_split = 0
    for f in nc.m.functions:
        for b in f.blocks:
            insts = list(b.instructions)
            out = []
            changed = False
            for inst in insts:
                si = inst.sync_info
                waits = list(si.on_wait) if si is not None else []
                if len(waits) > max_waits:
                    changed = True
                    for i, w in enumerate(waits[max_waits:]):
                        nop = mybir.InstNoOp(
                            name=f"wsp_{inst.name}_{i}", ins=[], outs=[])
                        nop.engine = inst.engine
                        nop.sync_info = bass_rust.SyncInfo(
                            on_wait=[w], on_update=[])
                        out.append(nop)
                        n_split += 1
                    inst.sync_info = bass_rust.SyncInfo(
                        on_wait=waits[:max_waits], on_update=list(si.on_update))
                out.append(inst)
            if changed:
                b.instructions = out
    return n_split


def _pad_w5(w5):
    """(1,64,4,4) -> (4,4,64,32) with real weights in out-column 0."""
    t = np.zeros((4, 4, 64, 32), np.float32)
    t[:, :, :, 0:1] = w5.transpose(2, 3, 1, 0)
    return np.ascontiguousarray(t)


def _col_major_pad(v, ncols):
    """(n,) -> (128, ncols) with element m at [m % 128, m // 128], zero pad."""
    out = np.zeros(128 * ncols, np.float32)
    out[: v.shape[0]] = v
    return np.ascontiguousarray(out.reshape(ncols, 128).T)


def build_program(ser_groups=SER_G, n_blocks=None, n_m3=N_M3, tail=TAIL,
                  with_scan=True, timing=False):
    if n_blocks is None:
        n_blocks = (T_FULL - tail) // KB - ser_groups
    assert ser_groups % 2 == 0, "sign-tile parity assumes even ser_groups"
    T = (ser_groups + n_blocks) * KB + tail
    nc = bass.Bass()

    # ---- inputs ----
    x_cols = nc.declare_dram_parameter("x_cols", [128, 16], f32, isOutput=False)
    win_t = nc.declare_dram_parameter("win_t", [2048, MROWS_A], f32, isOutput=False)
    bin_c = nc.declare_dram_parameter("bin_c", [128, 13], f32, isOutput=False)
    w1t = nc.declare_dram_parameter("w1t", [4, 4, 512, 512], f32, isOutput=False)
    w2t = nc.declare_dram_parameter("w2t", [4, 4, 512, 256], f32, isOutput=False)
    w3t = nc.declare_dram_parameter("w3t", [4, 4, 256, 128], f32, isOutput=False)
    w4t = nc.declare_dram_parameter("w4t", [4, 4, 128, 64], f32, isOutput=False)
    w5t = nc.declare_dram_parameter("w5t", [4, 4, 64, 32], f32, isOutput=False)
    g_all = nc.declare_dram_parameter("g_all", [128, 8], f32, isOutput=False)
    be_all = nc.declare_dram_parameter("be_all", [128, 8], f32, isOutput=False)
    wd2_t = nc.declare_dram_parameter("wd2_t", [6400, MROWS_C], f32, isOutput=False)
    bd2_c = nc.declare_dram_parameter("bd2_c", [128, 5], f32, isOutput=False)
    s0_in = nc.declare_dram_parameter("s0", [N, 1], f32, isOutput=False)
    ident_in = nc.declare_dram_parameter("ident", [128, 128], f32, isOutput=False)
    mtri_in = nc.declare_dram_parameter("mtri", [128, 128], f32, isOutput=False)
    ones_in = nc.declare_dram_parameter("ones", [128, 128], f32, isOutput=False)
    sgne_in = nc.declare_dram_parameter("sgne", [KB, N], f32, isOutput=False)
    sgno_in = nc.declare_dram_parameter("sgno", [KB, N], f32, isOutput=False)
    if with_scan:
        out_traj = nc.declare_dram_parameter("out", [T, N], f32, isOutput=True)
    else:
        w_out = nc.declare_dram_parameter("w_out", [N, N], f32, isOutput=True)

    # ---- internal DRAM ----
    h_shard = nc.dram_tensor("h_shard", [MROWS_A], f32)
    h_full = nc.dram_tensor("h_full", [NCORES * MROWS_A], f32, addr_space="Shared")
    c_scr = nc.dram_tensor("c_scr", [32, 6400], f32)
    wd_shard = nc.dram_tensor("wd_shard", [MROWS_C], f32)
    w_full = nc.dram_tensor("w_full", [NCORES * MROWS_C], f32, addr_space="Shared")

    with TileContext(nc) as tc:
        # ================= Phase A: h = W_in @ x + b_in (sharded) ==========
        with (
            tc.tile_pool(name="a_const", bufs=1) as acp,
            tc.tile_pool(name="a_slab", bufs=2) as asp,
            tc.tile_pool(name="a_ps", bufs=1, space="PSUM") as aps,
        ):
            xc = acp.tile([128, 16], f32)
            nc.sync.dma_start(out=xc[:, :], in_=x_cols[:, :])
            bc = acp.tile([128, 13], f32)
            nc.sync.dma_start(out=bc[:, :], in_=bin_c[:, :])
            hc = acp.tile([128, 13], f32)
            for jlo, jhi in ((0, 8), (8, 13)):
                ptiles = {}
                for j in range(jlo, jhi):
                    pt = aps.tile([128, 1], f32, tag=f"hps{j - jlo}", name=f"hps{j}")
                    ptiles[j] = pt
                for k in range(16):
                    gw = min(128 * jhi, MROWS_A) - 128 * jlo
                    slab = asp.tile([128, 1024], f32, tag="aslab")
                    nc.sync.dma_start(
                        out=slab[:, :gw],
                        in_=win_t[128 * k : 128 * (k + 1),
                                  128 * jlo : 128 * jlo + gw])
                    for j in range(jlo, jhi):
                        cj = 128 if j < 12 else 64
                        jj = j - jlo
                        nc.tensor.matmul(
                            ptiles[j][:cj, :],
                            slab[:, 128 * jj : 128 * jj + cj],
                            xc[:, k : k + 1],
                            start=(k == 0),
                            stop=(k == 15),
                        )
                for j in range(jlo, jhi):
                    cj = 128 if j < 12 else 64
                    nc.vector.tensor_tensor(
                        out=hc[:cj, j : j + 1], in0=ptiles[j][:cj, :],
                        in1=bc[:cj, j : j + 1], op=OP.add)
            for j in range(13):
                cj = 128 if j < 12 else 64
                nc.sync.dma_start(
                    out=h_shard[128 * j : 128 * j + cj], in_=hc[:cj, j])
        if timing:
            # timing build (TimelineSim is single-core): local DMA stand-in;
            # the analytic collective cost is added by the caller.
            nc.sync.dma_start(out=h_full[0:MROWS_A], in_=h_shard[:])
        else:
            nc.gpsimd.collective_compute(
                "AllGather", OP.bypass, replica_groups=[list(range(NCORES))],
                ins=[h_shard[:]], outs=[h_full[:]])

        # ================= Phase B: conv stack (replicated) ================
        _lvl = 9  # all conv layers (bisection gates left in place, fully on)
        h2d = h_full.rearrange("(c hw) -> c hw", hw=25)
        gsl = {1: (0, 4), 2: (4, 2), 3: (6, 1), 4: (7, 1)}  # (col offset, ncols)

        with (
            tc.tile_pool(name="bn_const", bufs=1) as bnp,
            tc.tile_pool(name="conv_ps", bufs=1, space="PSUM") as bps,
        ):
            g_sb = bnp.tile([128, 8], f32)
            nc.sync.dma_start(out=g_sb[:, :], in_=g_all[:, :])
            be_sb = bnp.tile([128, 8], f32)
            nc.sync.dma_start(out=be_sb[:, :], in_=be_all[:, :])

            def bn_relu(raw, hw, cch, lidx, j, out_ap):
                """BatchNorm(train) + ReLU from raw (cch,hw) into out_ap."""
                with tc.tile_pool(name=f"bn{lidx}_{j}", bufs=1) as p:
                    s1 = p.tile([cch, 1], f32, tag="s1")
                    nc.vector.tensor_reduce(s1[:, :], raw, axis=AX.X, op=OP.add)
                    mean = p.tile([cch, 1], f32, tag="mean")
                    nc.vector.tensor_scalar_mul(mean[:, :], s1[:, :], 1.0 / hw)
                    sq = p.tile([cch, hw], f32, tag="sq")
                    nc.vector.tensor_tensor(out=sq[:, :], in0=raw, in1=raw, op=OP.mult)
                    s2 = p.tile([cch, 1], f32, tag="s2")
                    nc.vector.tensor_reduce(s2[:, :], sq[:, :], axis=AX.X, op=OP.add)
                    ex2 = p.tile([cch, 1], f32, tag="ex2")
                    nc.vector.tensor_scalar_mul(ex2[:, :], s2[:, :], 1.0 / hw)
                    msq = p.tile([cch, 1], f32, tag="msq")
                    nc.vector.tensor_tensor(
                        out=msq[:, :], in0=mean[:, :], in1=mean[:, :], op=OP.mult)
                    var = p.tile([cch, 1], f32, tag="var")
                    nc.vector.tensor_tensor(
                        out=var[:, :], in0=ex2[:, :], in1=msq[:, :], op=OP.subtract)
                    vps = p.tile([cch, 1], f32, tag="vps")
                    nc.vector.tensor_scalar_add(vps[:, :], var[:, :], EPS)
                    sd = p.tile([cch, 1], f32, tag="sd")
                    nc.scalar.activation(sd[:, :], vps[:, :], AF.Sqrt)
                    rstd = p.tile([cch, 1], f32, tag="rstd")
                    nc.vector.reciprocal(rstd[:, :], sd[:, :])
                    co, _ = gsl[lidx]
                    scale = p.tile([cch, 1], f32, tag="scale")
                    nc.vector.tensor_tensor(
                        out=scale[:, :], in0=g_sb[:cch, co + j : co + j + 1],
                        in1=rstd[:, :], op=OP.mult)
                    t1 = p.tile([cch, 1], f32, tag="t1")
                    nc.vector.tensor_tensor(
                        out=t1[:, :], in0=mean[:, :], in1=scale[:, :], op=OP.mult)
                    bia = p.tile([cch, 1], f32, tag="bia")
                    nc.vector.tensor_tensor(
                        out=bia[:, :], in0=be_sb[:cch, co + j : co + j + 1],
                        in1=t1[:, :], op=OP.subtract)
                    nc.scalar.activation(
                        out_ap, raw, AF.Relu, bias=bia[:, :], scale=scale[:, :])

            # ---- L1: up2(h:512x5x5)->512x10x10 conv 512->512 ----
            with (
                tc.tile_pool(name="l1_in", bufs=1) as l1i,
                tc.tile_pool(name="l1_w", bufs=2) as l1w,
                tc.tile_pool(name="l1_out", bufs=1) as l1o,
            ):
                pads1 = []
                for j in range(4):
                    hm = l1i.tile([128, 25], f32, tag=f"hm{j}")
                    nc.sync.dma_start(out=hm[:, :], in_=h2d[128 * j : 128 * (j + 1), :])
                    pad = l1i.tile([128, 13 * 13], f32, tag=f"pad1_{j}")
                    nc.vector.memset(pad[:, :], 0.0)
                    pv = pad[:, :].rearrange("c (h w) -> c h w", h=13)
                    hv = hm[:, :].rearrange("c (h w) -> c h w", h=5)
                    for a in range(2):
                        for b in range(2):
                            nc.vector.tensor_copy(
                                pv[:, a + 1 : a + 11 : 2, b + 1 : b + 11 : 2], hv[:, :, :])
                    pads1.append(pad)
                ps1s = []
                for jo in range(4):
                    p1 = bps.tile([128, 100], f32, tag=f"l1ps{jo}", name=f"l1ps{jo}")
                    ps1s.append(p1)
                nmm = 0
                for ji in range(4):
                    for dy in range(4):
                        for dx in range(4):
                            slab = l1w.tile([128, 512], f32, tag="w1slab")
                            nc.sync.dma_start(
                                out=slab[:, :],
                                in_=w1t[dy, dx, 128 * ji : 128 * (ji + 1), :])
                            rhs = pads1[ji][:, :].rearrange(
                                "c (h w) -> c h w", h=13)[:, dy : dy + 10, dx : dx + 10]
                            for jo in range(4):
                                nc.tensor.matmul(
                                    ps1s[jo][:, :],
                                    slab[:, 128 * jo : 128 * (jo + 1)], rhs,
                                    start=(nmm == 0), stop=(nmm == 63))
                            nmm += 1
                pads2 = []
                for jo in range(4):
                    raw = l1o.tile([128, 100], f32, tag=f"raw1_{jo}")
                    nc.vector.tensor_copy(raw[:, :], ps1s[jo][:, :])
                    relu = l1o.tile([128, 100], f32, tag=f"relu1_{jo}")
                    bn_relu(raw[:, :], 100, 128, 1, jo, relu[:, :])
                    pad = l1o.tile([128, 23 * 23], f32, tag=f"pad2_{jo}")
                    nc.vector.memset(pad[:, :], 0.0)
                    pv = pad[:, :].rearrange("c (h w) -> c h w", h=23)
                    rv = relu[:, :].rearrange("c (h w) -> c h w", h=10)
                    for a in range(2):
                        for b in range(2):
                            nc.vector.tensor_copy(
                                pv[:, a + 1 : a + 21 : 2, b + 1 : b + 21 : 2], rv[:, :, :])
                    pads2.append(pad)

                if _lvl >= 2:
                  # ---- L2: 512x20x20 conv 512->256 ----
                  with (
                      tc.tile_pool(name="l2_w", bufs=2) as l2w,
                      tc.tile_pool(name="l2_out", bufs=1) as l2o,
                  ):
                      psA = bps.tile([128, 400], f32, tag="cpsA")
                      psB = bps.tile([128, 400], f32, tag="cpsB")
                      nmm = 0
                      for ji in range(4):
                          for dy in range(4):
                              for dx in range(4):
                                  slab = l2w.tile([128, 256], f32, tag="w2slab")
                                  nc.sync.dma_start(
                                      out=slab[:, :],
                                      in_=w2t[dy, dx, 128 * ji : 128 * (ji + 1), :])
                                  rhs = pads2[ji][:, :].rearrange(
                                      "c (h w) -> c h w", h=23)[:, dy : dy + 20, dx : dx + 20]
                                  nc.tensor.matmul(
                                      psA[:, :], slab[:, 0:128], rhs,
                                      start=(nmm == 0), stop=(nmm == 63))
                                  nc.tensor.matmul(
                                      psB[:, :], slab[:, 128:256], rhs,
                                      start=(nmm == 0), stop=(nmm == 63))
                                  nmm += 1
                      pads3 = []
                      for jo, ps in enumerate((psA, psB)):
                          raw = l2o.tile([128, 400], f32, tag=f"raw2_{jo}")
                          nc.vector.tensor_copy(raw[:, :], ps[:, :])
                          relu = l2o.tile([128, 400], f32, tag=f"relu2_{jo}")
                          bn_relu(raw[:, :], 400, 128, 2, jo, relu[:, :])
                          pad = l2o.tile([128, 43 * 43], f32, tag=f"pad3_{jo}")
                          nc.vector.memset(pad[:, :], 0.0)
                          pv = pad[:, :].rearrange("c (h w) -> c h w", h=43)
                          rv = relu[:, :].rearrange("c (h w) -> c h w", h=20)
                          for a in range(2):
                              for b in range(2):
                                  nc.vector.tensor_copy(
                                      pv[:, a + 1 : a + 41 : 2, b + 1 : b + 41 : 2],
                                      rv[:, :, :])
                          pads3.append(pad)

                      if _lvl >= 3:
                        # ---- L3: 256x40x40 conv 256->128 ----
                        with (
                            tc.tile_pool(name="l3_w", bufs=1) as l3w,
                            tc.tile_pool(name="l3_out", bufs=1) as l3o,
                        ):
                            wsl3 = l3w.tile([128, 32 * 128], f32)
                            for ji in range(2):
                                for dy in range(4):
                                    for dx in range(4):
                                        si = (ji * 16 + dy * 4 + dx) * 128
                                        nc.sync.dma_start(
                                            out=wsl3[:, si : si + 128],
                                            in_=w3t[dy, dx, 128 * ji : 128 * (ji + 1), :])
                            raw3 = l3o.tile([128, 1600], f32)
                            for st in range(4):
                                ps = bps.tile([128, 400], f32, tag="cps", bufs=2)
                                nmm = 0
                                for ji in range(2):
                                    for dy in range(4):
                                        for dx in range(4):
                                            si = (ji * 16 + dy * 4 + dx) * 128
                                            rhs = pads3[ji][:, :].rearrange(
                                                "c (h w) -> c h w", h=43)[
                                                :, st * 10 + dy : st * 10 + dy + 10,
                                                dx : dx + 40]
                                            nc.tensor.matmul(
                                                ps[:, :], wsl3[:, si : si + 128], rhs,
                                                start=(nmm == 0), stop=(nmm == 31))
                                            nmm += 1
                                nc.vector.tensor_copy(
                                    raw3[:, 400 * st : 400 * (st + 1)], ps[:, :])
                            relu3 = l3o.tile([128, 1600], f32)
                            bn_relu(raw3[:, :], 1600, 128, 3, 0, relu3[:, :])
                            pad4 = l3o.tile([128, 83 * 83], f32)
                            nc.vector.memset(pad4[:, :], 0.0)
                            pv = pad4[:, :].rearrange("c (h w) -> c h w", h=83)
                            rv = relu3[:, :].rearrange("c (h w) -> c h w", h=40)
                            for a in range(2):
                                for b in range(2):
                                    nc.vector.tensor_copy(
                                        pv[:, a + 1 : a + 81 : 2, b + 1 : b + 81 : 2],
                                        rv[:, :, :])

                            if _lvl >= 4:
                              # ---- L4: 128x80x80 conv 128->64 ----
                              with (
                                  tc.tile_pool(name="l4_w", bufs=1) as l4w,
                                  tc.tile_pool(name="l4_out", bufs=1) as l4o,
                              ):
                                  wsl4 = l4w.tile([128, 16 * 64], f32)
                                  for dy in range(4):
                                      for dx in range(4):
                                          si = (dy * 4 + dx) * 64
                                          nc.sync.dma_start(
                                              out=wsl4[:, si : si + 64],
                                              in_=w4t[dy, dx, :, :])
                                  raw4 = l4o.tile([64, 6400], f32)
                                  for st in range(16):
                                      ps = bps.tile([64, 400], f32, tag="cps", bufs=2)
                                      nmm = 0
                                      for dy in range(4):
                                          for dx in range(4):
                                              si = (dy * 4 + dx) * 64
                                              rhs = pad4[:, :].rearrange(
                                                  "c (h w) -> c h w", h=83)[
                                                  :, st * 5 + dy : st * 5 + dy + 5,
                                                  dx : dx + 80]
                                              nc.tensor.matmul(
                                                  ps[:, :], wsl4[:, si : si + 64], rhs,
                                                  start=(nmm == 0), stop=(nmm == 15))
                                              nmm += 1
                                      nc.vector.tensor_copy(
                                          raw4[:, 400 * st : 400 * (st + 1)], ps[:, :])
                                  pad5 = l4o.tile([64, 83 * 83], f32)
                                  nc.vector.memset(pad5[:, :], 0.0)
                                  pv5 = pad5[:, :].rearrange("c (h w) -> c h w", h=83)[
                                      :, 1:81, 1:81]
                                  bn_relu(raw4[:, :], 6400, 64, 4, 0, pv5)

                                  if _lvl >= 5:
                                    # ---- L5: 64x80x80 conv 64->1 + tanh -> c ----
                                    with (
                                        tc.tile_pool(name="l5_w", bufs=1) as l5w,
                                        tc.tile_pool(name="l5_out", bufs=1) as l5o,
                                    ):
                                        wsl5 = l5w.tile([64, 16 * 32], f32)
                                        for dy in range(4):
                                            for dx in range(4):
                                                _p5 = (dy * 4 + dx) * 32
                                                nc.sync.dma_start(
                                                    out=wsl5[:, _p5 : _p5 + 32],
                                                    in_=w5t[dy, dx, :, :])
                                        for st in range(16):
                                            ps = bps.tile([32, 400], f32, tag="cps", bufs=2)
                                            nmm = 0
                                            for dy in range(4):
                                                for dx in range(4):
                                                    rhs = pad5[:, :].rearrange(
                                                        "c (h w) -> c h w", h=83)[
                                                        :, st * 5 + dy : st * 5 + dy + 5,
                                                        dx : dx + 80]
                                                    _p5 = (dy * 4 + dx) * 32
                                                    nc.tensor.matmul(
                                                        ps[:, :],
                                                        wsl5[:, _p5 : _p5 + 32],
                                                        rhs,
                                                        start=(nmm == 0), stop=(nmm == 15))
                                                    nmm += 1
                                            c32 = l5o.tile([32, 400], f32, tag="c32", name=f"c32_{st}")
                                            nc.scalar.activation(c32[:, :], ps[:, :], AF.Tanh)
                                            nc.sync.dma_start(
                                                out=c_scr[:, 400 * st : 400 * (st + 1)], in_=c32[:, :])

        # ================= Phase C: w = W_d2 @ c + b_d2 (sharded) ==========
        _skip_c = False
        if not _skip_c:
          with (
              tc.tile_pool(name="c_const", bufs=1) as ccp,
              tc.tile_pool(name="c_slab", bufs=2) as csp,
              tc.tile_pool(name="c_ps", bufs=1, space="PSUM") as cps,
          ):
              c_cols = ccp.tile([128, 50], f32)
              nc.sync.dma_start(
                  out=c_cols[:, :], in_=c_scr[0, :].rearrange("(f p) -> p f", p=128))
              bdc = ccp.tile([128, 5], f32)
              nc.sync.dma_start(out=bdc[:, :], in_=bd2_c[:, :])
              wtiles = {}
              for j in range(5):
                  wt_ps = cps.tile([128, 1], f32, tag=f"wps{j}", name=f"wps{j}")
                  wtiles[j] = wt_ps
              for k in range(50):
                  slab = csp.tile([128, MROWS_C], f32, tag="cslab")
                  nc.sync.dma_start(
                      out=slab[:, :], in_=wd2_t[128 * k : 128 * (k + 1), :])
                  for j in range(5):
                      cj = 128 if j < 4 else 84
                      nc.tensor.matmul(
                          wtiles[j][:cj, :], slab[:, 128 * j : 128 * j + cj],
                          c_cols[:, k : k + 1], start=(k == 0), stop=(k == 49))
              wdc = ccp.tile([128, 5], f32)
              for j in range(5):
                  cj = 128 if j < 4 else 84
                  nc.vector.tensor_tensor(
                      out=wdc[:cj, j : j + 1], in0=wtiles[j][:cj, :],
                      in1=bdc[:cj, j : j + 1], op=OP.add)
              for j in range(5):
                  cj = 128 if j < 4 else 84
                  nc.sync.dma_start(
                      out=wd_shard[128 * j : 128 * j + cj], in_=wdc[:cj, j])
        if not _skip_c:
            if timing:
                nc.sync.dma_start(out=w_full[0:MROWS_C], in_=wd_shard[:])
            else:
                nc.gpsimd.collective_compute(
                    "AllGather", OP.bypass, replica_groups=[list(range(NCORES))],
                    ins=[wd_shard[:]], outs=[w_full[:]])

        if not with_scan:
            with tc.tile_pool(name="wout", bufs=1) as wop:
                w_sb0 = wop.tile([N, N], f32)
                nc.sync.dma_start(
                    out=w_sb0[:, :],
                    in_=w_full[0 : N * N].rearrange("(j i) -> j i", i=N))
                nc.sync.dma_start(out=w_out[:, :], in_=w_sb0[:, :])

        # ================= Phase D: spiking scan =========================
        if with_scan:
          ms = [3] * min(n_m3, n_blocks) + [2] * max(0, n_blocks - n_m3)
          kbs = [KB] * n_blocks
          if tail:
              ms.append(3)
              kbs.append(tail)
          nbt = len(ms)           # picard blocks incl tail
          ser_steps = ser_groups * KB
          with (
              tc.tile_pool(name="d_const", bufs=1) as dcp,
              tc.tile_pool(name="d_sb", bufs=2) as dsb,
          ):
            w_sb = dcp.tile([N, N], f32)
            nc.sync.dma_start(
                out=w_sb[:, :],
                in_=w_full[0 : N * N].rearrange("(j i) -> j i", i=N))
            wneg = dcp.tile([N, N], f32)
            nc.vector.tensor_scalar_mul(wneg[:, :], w_sb[:, :], -1.0)
            mtri = dcp.tile([128, 128], f32)
            nc.sync.dma_start(out=mtri[:, :], in_=mtri_in[:, :])
            ident = dcp.tile([128, 128], f32)
            nc.sync.dma_start(out=ident[:, :], in_=ident_in[:, :])
            onesm = dcp.tile([128, 128], f32)
            nc.sync.dma_start(out=onesm[:, :], in_=ones_in[:, :])
            sgne = dcp.tile([KB, N], f32)
            nc.sync.dma_start(out=sgne[:, :], in_=sgne_in[:, :])
            sgno = dcp.tile([KB, N], f32)
            nc.sync.dma_start(out=sgno[:, :], in_=sgno_in[:, :])
            s0c = dcp.tile([N, 1], f32)
            nc.sync.dma_start(out=s0c[:, :], in_=s0_in[:, :])

            # picard block state tiles (allocated up front so the serial
            # handoff can fill block 0 before the serial PSUM pool closes)
            ms = list(ms)
            ub_t = [None] * nbt
            s0_t = [None] * nbt          # [1,N] start-state rows (partition 0)
            st_t = [None] * nbt
            ub_t[0] = dsb.tile([KB, N], f32, tag="ub", bufs=3, name="ub0")
            s0_t[0] = dsb.tile([1, N], f32, tag="s0row", bufs=3, name="s0r0")

            # ---------- serial phase: 2-op steps, t-major output ----------
            # ub_ser: cols 0..126 = u' of the group, col 127 = group start s'
            serial_psum = tc.tile_pool(name="d_ps_ser", bufs=2, space="PSUM")
            dps = serial_psum.__enter__()
            ub_ser = dcp.tile([N, 128], f32)
            nc.vector.tensor_copy(ub_ser[:, 127:128], s0c[:, :])
            ybank = dps.tile([N, 1], f32, tag="ybank", name="ybank", bufs=1)
            nc.tensor.matmul(
                ybank[:, :], w_sb[:, :], s0c[:, :], start=True, stop=True)

            ser_dma_grp = 4       # serial groups per output DMA
            obufS = None
            last_ubT = None       # SBUF [128,N] of the last serial group
            last_stS = None       # PSUM [128,N] finish of the last group
            for g in range(ser_groups):
                for k in range(KB):
                    nc.scalar.activation(
                        ub_ser[:, k : k + 1], ybank[:, :], AF.Tanh)
                    nc.tensor.matmul(
                        ybank[:, :], wneg[:, :], ub_ser[:, k : k + 1],
                        start=False, stop=True, skip_group_check=True)
                # group recon: transpose -> prefix -> sign -> batched DMA
                ubT_ps = dps.tile([128, N], f32, tag="ubTps")
                nc.tensor.transpose(ubT_ps[:, :], ub_ser[:, :], ident[:N, :N])
                ubT_sb = dsb.tile([128, N], f32, tag="ubTsb")
                nc.vector.tensor_copy(ubT_sb[:, :], ubT_ps[:, :])
                stS_ps = dps.tile([KB, N], f32, tag="stSps")
                nc.tensor.matmul(
                    stS_ps[:, :], mtri[:, 1 : KB + 1], ubT_sb[:, :],
                    start=True, stop=True)
                gi = g % ser_dma_grp
                if gi == 0:
                    obufS = dsb.tile([KB, ser_dma_grp * N], f32, tag="obufS")
                sg = sgne if g % 2 == 0 else sgno
                nc.vector.tensor_tensor(
                    out=obufS[:, gi * N : (gi + 1) * N],
                    in0=stS_ps[:, :], in1=sg[:, :], op=OP.mult)
                ngrp = min(ser_dma_grp, ser_groups - (g - gi))
                if gi == ngrp - 1:
                    r0 = (g - gi) * KB
                    nc.sync.dma_start(
                        out=out_traj[r0 : r0 + ngrp * KB, :].rearrange(
                            "(b p) n -> p b n", b=ngrp),
                        in_=obufS[:, : ngrp * N].rearrange(
                            "p (b n) -> p b n", b=ngrp))
                # next start column via single-col prefix matmul
                scol_ps = dps.tile([N, 1], f32, tag="scolps")
                nc.tensor.matmul(
                    scol_ps[:, :], ubT_sb[:, :], mtri[:, 127:128],
                    start=True, stop=True)
                if g < ser_groups - 1:
                    nc.vector.tensor_copy(ub_ser[:, 127:128], scol_ps[:, :])
                else:
                    last_ubT = ubT_sb
                    last_scol = scol_ps

            # exact serial -> block-0 handoff (inside the serial PSUM
            # scope: last_scol is a PSUM tile). The end-state column is
            # transposed into a partition-0 row.
            nc.gpsimd.tensor_copy(ub_t[0][:, :], last_ubT[0:KB, :])
            scol_sb = dsb.tile([N, 1], f32, tag="scolsb")
            nc.vector.tensor_copy(scol_sb[:, :], last_scol[:, :])
            s0T_ps = dps.tile([1, N], f32, tag="s0Tps", bufs=1)
            nc.tensor.transpose(s0T_ps[:, :], scol_sb[:, :], ident[:N, :N])
            nc.vector.tensor_copy(s0_t[0][:, :], s0T_ps[:, :])
            serial_psum.__exit__(None, None, None)

            # ---------- pipelined blocked-Picard phase (LIT) ----------
            # Per block b: seeds + preview start from block b-1's U^{(M-1)}
            # (sum via e127 matmul); only the last iteration waits for the
            # true (converged) start state of the block.
            picard_psum = tc.tile_pool(name="d_ps_pic", bufs=2, space="PSUM")
            dps = picard_psum.__enter__()
            obuf = None
            obuf_base = 0

            def emit_iter(b):
                """One Picard iteration of block b (mm1 pair,copy,mm2,tanh).
                S' cols = u-row prefix (strict-lower mtri) + s0 broadcast."""
                kb = kbs[b]
                sp_ps = dps.tile([N, 128], f32, tag="spps")
                nc.tensor.matmul(
                    sp_ps[:, :kb], ub_t[b][:, :], mtri[0:KB, :kb],
                    start=True, stop=False)
                nc.tensor.matmul(
                    sp_ps[:, :kb], s0_t[b][:, :], onesm[0:1, :kb],
                    start=False, stop=True)
                sp_sb = dsb.tile([N, KB], f32, tag="spsb")
                nc.vector.tensor_copy(sp_sb[:, :kb], sp_ps[:, :kb])
                y_ps = dps.tile([KB, N], f32, tag="yps")
                nc.tensor.matmul(
                    y_ps[:kb, :], sp_sb[:, :kb], w_sb[:, :],
                    start=True, stop=True)
                nc.scalar.activation(ub_t[b][0:kb, :], y_ps[:kb, :], AF.Tanh)

            for s in range(nbt + 1):
                # --- pre-ops of block s (reads block s-1's U^{(M-1)}) ---
                if 0 < s < nbt:
                    ub_t[s] = dsb.tile([KB, N], f32, tag="ub", bufs=3,
                                       name=f"ub{s}")
                    s0_t[s] = dsb.tile([1, N], f32, tag="s0row", bufs=3,
                                       name=f"s0r{s}")
                    if True:
                        sum_ps = dps.tile([1, N], f32, tag="sumps")
                        nc.tensor.matmul(
                            sum_ps[:, :], onesm[0:KB, 0:1], ub_t[s - 1][:, :],
                            start=True, stop=True)
                        nc.vector.tensor_tensor(
                            out=s0_t[s][:, :], in0=s0_t[s - 1][:, :],
                            in1=sum_ps[:, :], op=OP.subtract)
                        nc.gpsimd.tensor_copy(
                            ub_t[s][:, :], ub_t[s - 1][:, :])
                # --- final iteration of block s-1 ---
                if s > 0:
                    emit_iter(s - 1)
                # --- iterations 1..M-1 of block s (preview start) ---
                if s < nbt:
                    for m in range(1, ms[s]):
                        emit_iter(s)
                # --- true-start handoff for block s (after tanh(s-1, M),
                #     before the final iteration emitted next stage) ---
                if 0 < s < nbt:
                    sum2_ps = dps.tile([1, N], f32, tag="sumps",
                                       name=f"sum2_{s}")
                    nc.tensor.matmul(
                        sum2_ps[:, :], onesm[0:KB, 0:1], ub_t[s - 1][:, :],
                        start=True, stop=True)
                    nc.vector.tensor_tensor(
                        out=s0_t[s][:, :], in0=s0_t[s - 1][:, :],
                        in1=sum2_ps[:, :], op=OP.subtract)
                # --- finish + output of block s-1 ---
                if s > 0:
                    b = s - 1
                    kb = kbs[b]
                    st_t[b] = dps.tile([KB, N], f32, tag="stps",
                                       name=f"st{b}")
                    nc.tensor.matmul(
                        st_t[b][:kb, :], mtri[0:KB, 1 : kb + 1], ub_t[b][:, :],
                        start=True, stop=False)
                    nc.tensor.matmul(
                        st_t[b][:kb, :], onesm[0:1, 0:kb], s0_t[b][:, :],
                        start=False, stop=True)
                    if b < n_blocks:
                        gi = b % DMA_GRP
                        if gi == 0:
                            obuf = dsb.tile([KB, DMA_GRP * N], f32,
                                            tag="obuf")
                            obuf_base = b
                        sg = sgne if b % 2 == 0 else sgno
                        nc.vector.tensor_tensor(
                            out=obuf[:, gi * N : (gi + 1) * N],
                            in0=st_t[b][:, :], in1=sg[:, :],
                            op=OP.mult)
                        ngrp = min(DMA_GRP, n_blocks - obuf_base)
                        if gi == ngrp - 1:
                            r0 = ser_steps + obuf_base * KB
                            nc.sync.dma_start(
                                out=out_traj[
                                    r0 : r0 + ngrp * KB, :].rearrange(
                                    "(b p) n -> p b n", b=ngrp),
                                in_=obuf[:, : ngrp * N].rearrange(
                                    "p (b n) -> p b n", b=ngrp))
                    else:
                        # tail block: sign pattern continues the parity of
                        # block index b (tail rows start at an odd offset)
                        sg = sgne if b % 2 == 0 else sgno
                        otail = dsb.tile([KB, N], f32, tag="otail")
                        nc.vector.tensor_tensor(
                            out=otail[:kb, :], in0=st_t[b][:kb, :],
                            in1=sg[:kb, :], op=OP.mult)
                        nc.sync.dma_start(
                            out=out_traj[T - kb : T, :], in_=otail[:kb, :])
            picard_psum.__exit__(None, None, None)

    return nc


def _marshal_inputs(inputs):
    """Build the 8 per-core input maps from the full problem inputs."""
    x = np.asarray(inputs["x"], np.float32).reshape(2048)
    win = np.asarray(inputs["W_in"], np.float32)
    b_in = np.asarray(inputs["b_in"], np.float32)
    wd2 = np.asarray(inputs["W_d2"], np.float32)
    bd2 = np.asarray(inputs["b_d2"], np.float32)
    sp = np.asarray(inputs["start_part"], np.float32)

    x_cols = np.ascontiguousarray(x.reshape(16, 128).T)
    g_all = np.zeros((128, 8), np.float32)
    be_all = np.zeros((128, 8), np.float32)
    g_all[:, 0:4] = _col_major_pad(np.asarray(inputs["g1"], np.float32), 4)
    g_all[:, 4:6] = _col_major_pad(np.asarray(inputs["g2"], np.float32), 2)
    g_all[:, 6:7] = _col_major_pad(np.asarray(inputs["g3"], np.float32), 1)
    g_all[:, 7:8] = _col_major_pad(np.asarray(inputs["g4"], np.float32), 1)
    be_all[:, 0:4] = _col_major_pad(np.asarray(inputs["be1"], np.float32), 4)
    be_all[:, 4:6] = _col_major_pad(np.asarray(inputs["be2"], np.float32), 2)
    be_all[:, 6:7] = _col_major_pad(np.asarray(inputs["be3"], np.float32), 1)
    be_all[:, 7:8] = _col_major_pad(np.asarray(inputs["be4"], np.float32), 1)
    wts = {
        "w1t": np.ascontiguousarray(
            np.asarray(inputs["w1"], np.float32).transpose(2, 3, 1, 0)),
        "w2t": np.ascontiguousarray(
            np.asarray(inputs["w2"], np.float32).transpose(2, 3, 1, 0)),
        "w3t": np.ascontiguousarray(
            np.asarray(inputs["w3"], np.float32).transpose(2, 3, 1, 0)),
        "w4t": np.ascontiguousarray(
            np.asarray(inputs["w4"], np.float32).transpose(2, 3, 1, 0)),
        "w5t": _pad_w5(np.asarray(inputs["w5"], np.float32)),
    }
    s0 = np.ascontiguousarray(sp[-1].reshape(N, 1))
    ident = np.eye(128, dtype=np.float32)
    # prefix matrix: S'[i,t] = sum_k ubT[k,i]*mtri[k,t]; strict-lower -1s
    # for the u' rows, +1 base row (127) for the s'0 term.
    mtri = np.zeros((128, 128), np.float32)
    for k in range(127):
        mtri[k, k + 1 :] = -1.0
    mtri[127, :] = 1.0
    # all-ones helper (column sums / base-row broadcasts via matmul)
    onesm = np.ones((128, 128), np.float32)
    # unpriming signs by output row parity: out[t] = (-1)^(t+1) s'_{t+1};
    # within a group starting at even global t, row j gets (-1)^(j+1).
    sgne = np.tile(
        np.where(np.arange(KB) % 2 == 0, -1.0, 1.0
                 ).astype(np.float32)[:, None], (1, N))
    sgno = -sgne

    wd2_pad = np.zeros((NCORES * MROWS_C, 6400), np.float32)
    wd2_pad[: wd2.shape[0]] = wd2
    bd2_pad = np.zeros(NCORES * MROWS_C, np.float32)
    bd2_pad[: bd2.shape[0]] = bd2

    in_maps = []
    for c in range(NCORES):
        m = {
            "x_cols": x_cols,
            "win_t": np.ascontiguousarray(
                win[MROWS_A * c : MROWS_A * (c + 1)].T),
            "bin_c": _col_major_pad(b_in[MROWS_A * c : MROWS_A * (c + 1)], 13),
            "g_all": g_all,
            "be_all": be_all,
            "wd2_t": np.ascontiguousarray(
                wd2_pad[MROWS_C * c : MROWS_C * (c + 1)].T),
            "bd2_c": _col_major_pad(bd2_pad[MROWS_C * c : MROWS_C * (c + 1)], 5),
            "s0": s0,
            "ident": ident,
            "mtri": mtri,
            "ones": onesm,
            "sgne": sgne,
            "sgno": sgno,
        }
        m.update(wts)
        in_maps.append(m)
    return in_maps


LAST_EXEC_NS = None


def kernel(**inputs) -> np.ndarray:
    global LAST_EXEC_NS
    import os

    trace = bool(os.environ.get("KERNEL_TRACE"))
    nc = build_program()
    _split_excess_waits(nc)
    in_maps = _marshal_inputs(inputs)
    res = run_bass_kernel_spmd(nc, in_maps, list(range(NCORES)), trace=trace)
    if res.exec_time_ns is not None:
        LAST_EXEC_NS = res.exec_time_ns
    out = np.asarray(res.results[0]["out"], np.float32)
    return out.reshape(1, T_FULL, N)


def _host_device_sim(w, s_init, ser_groups=SER_G, n_blocks=None, n_m3=N_M3,
                     tail=TAIL):
    """Numpy mirror of the device schedule (pipelined LIT semantics)."""
    if n_blocks is None:
        n_blocks = (T_FULL - tail) // KB - ser_groups
    ser_steps = ser_groups * KB
    T = ser_steps + n_blocks * KB + tail
    ms = [3] * min(n_m3, n_blocks) + [2] * max(0, n_blocks - n_m3)
    kbs = [KB] * n_blocks
    if tail:
        ms.append(3)
        kbs.append(tail)
    out_p = np.empty((T, N), np.float32)
    yp = (s_init @ w).astype(np.float32)
    sp = s_init.copy()
    ubh = np.zeros((KB, N), np.float32)
    for t in range(ser_steps):
        up = np.tanh(yp).astype(np.float32)
        ubh[t % KB] = up
        yp = (yp - (up @ w).astype(np.float32)).astype(np.float32)
        sp = (sp - up).astype(np.float32)
        out_p[t] = sp

    def prefix(s0, U, Kb):
        S = np.empty((Kb + 1, N), np.float32)
        S[0] = s0
        S[1:] = s0 - np.cumsum(U[:Kb], axis=0, dtype=np.float32)
        return S

    true_prev = out_p[ser_steps - 1].copy()   # true start of block 0 (exact)
    seeds = ubh.copy()                        # U^{(M-1)} of "block -1"
    t = ser_steps
    for b in range(len(ms)):
        M, kb = ms[b], kbs[b]
        if b == 0:
            start_all = true_prev             # exact for every iteration
            true_b = true_prev
        else:
            preview = (true_prev - seeds.sum(axis=0,
                                             dtype=np.float32)).astype(
                np.float32)
            start_all = preview
            true_b = None                     # filled after U^{(M)} known
        U = seeds.copy()
        for m in range(1, M):                 # preview iterations
            S = prefix(start_all, U, kb)
            Y = (S[:kb] @ w).astype(np.float32)
            U = np.tanh(Y).astype(np.float32)
        if b == 0:
            true_b = true_prev
        else:
            # sum2 over prev block's converged U^{(M)}
            true_b = (true_prev - prev_conv.sum(axis=0,
                                                dtype=np.float32)).astype(
                np.float32)
        seeds_next = U.copy()                 # U^{(M-1)} of this block
        S = prefix(true_b, U, kb)             # final iteration (true start)
        Y = (S[:kb] @ w).astype(np.float32)
        U = np.tanh(Y).astype(np.float32)
        St = prefix(true_b, U, kb)            # finish
        out_p[t : t + kb] = St[1 : kb + 1]
        prev_conv = U
        seeds = seeds_next
        true_prev = true_b
        t += kb
    tt = np.arange(T)[:, None]
    return out_p * np.where((tt + 1) % 2 == 0, 1.0, -1.0).astype(np.float32)


if __name__ == "__main__":
    # CoreSim selftest with a short schedule (no hardware needed).
    import sys
    import time

    SG, NB, NM3, TL = 2, 5, 2, 105
    T_test = (SG + NB) * KB + TL
    nc = build_program(ser_groups=SG, n_blocks=NB, n_m3=NM3, tail=TL)
    print("program built, T_test =", T_test, flush=True)

    sys.path.insert(0, "/root/problem")
    import jax
    jax.config.update("jax_platform_name", "cpu")
    import reference

    inputs = reference.setup_inputs()
    inputs = {k: np.asarray(v) for k, v in inputs.items()}
    in_maps = _marshal_inputs(inputs)

    from concourse.bass_interp import MultiCoreSim

    t0 = time.time()
    sim = MultiCoreSim(nc, NCORES)
    for i in range(NCORES):
        for k, v in in_maps[i].items():
            sim.cores[i].tensor(k)[:] = v
    sim.simulate()
    print("sim time", time.time() - t0, flush=True)
    got = np.array(sim.cores[0].tensor("out"))

    w = np.load("/tmp/w_host.npy").astype(np.float32)
    s_init = np.asarray(inputs["start_part"])[-1].astype(np.float32)
    exp = _host_device_sim(w, s_init, SG, NB, NM3, TL)
    err = np.abs(got - exp)
    print("vs host-device-sim: absmax", err.max(),
          "rel", np.linalg.norm(got - exp) / max(np.linalg.norm(exp), 1e-9))
    # also vs plain serial recurrence (informative)
    sref = s_init.copy()
    ser = np.empty((T_test, N), np.float32)
    for t in range(T_test):
        sref = (np.tanh((sref @ w).astype(np.float32)) - sref).astype(np.float32)
        ser[t] = sref
    d2 = got - ser
    print("vs plain serial: absmax", np.abs(d2).max(),
          "rel", np.linalg.norm(d2) / np.linalg.norm(ser))



# revision 32
# speedup vs baseline: 1.8705x; 1.8705x over previous
"""Trainium2 Bass kernel for nn_DCGAN_G (DCGAN generator + 69-neuron spiking scan).

Strategy (8 NeuronCores, SPMD):
  A. W_in matvec (12800x2048) row-sharded 8x -> AllGather h1 (12800).
  B. DCGAN conv stack replicated on every core (tiny: ~3 GMAC).
  C. W_d2 matvec (4761x6400) row-sharded 8x -> AllGather w (69x69).
  D. 99800-step spiking recurrence in "primed" coordinates
     s'_t = (-1)^t s_t (tanh odd => u'_t = tanh(s'_t @ w)):
       serial phase (3048 steps), y'-space 2-op steps:
         y'_{t+1} = y'_t - u'_t @ w  (PSUM-accumulating matmul + tanh);
         trajectory emitted t-major per 127-step group (PE transpose +
         prefix matmul + sign multiply + direct DMA to the output).
       pipelined blocked-Picard phase: 127-step blocks, M in {2,3}
       iterations of {S' = prefix(U', s'0) via matmul with a triangular
       constant; Y' = S'@w; U' = tanh(Y')}.  Software-pipelined across
       blocks with "last-iteration-true" (LIT) semantics: iterations
       1..M-1 use a previewed start state (prefix-sum of the previous
       block's U^{(M-1)} via a ones-column matmul), only the final
       iteration waits for the previous block's converged end state.
       Critical path per block = 6 engine ops instead of 4M+2.
     Outputs are produced t-major per block (finish matmul with the
     triangular constant -> sign multiply -> batched DMA straight into
     the (T,69) output), eliminating the i-major trajectory round-trip
     and final transpose pass entirely.
     Host-validated (exact op-order mirror) vs jax ref: rel ~ 1.0e-3.
"""
import numpy as np

import bass_rust
import concourse.bass as bass
import concourse.mybir as mybir
from concourse.bass_utils import run_bass_kernel_spmd
from concourse.tile import TileContext
from concourse.vector_clock import ScopedClock

f32 = mybir.dt.float32
AF = mybir.ActivationFunctionType
OP = mybir.AluOpType
AX = mybir.AxisListType

T_FULL = 99800
N = 69
NCORES = 8
EPS = 1e-5
MROWS_A = 1600        # W_in rows per core
MROWS_C = 596         # W_d2 rows per core (8*596=4768 >= 4761)
KB = 127              # picard block length / serial group length
SER_G = 24            # serial groups (24*127 = 3048 serial steps)
N_M3 = 54             # leading picard blocks run M=3; the rest M=2
TAIL = 105            # tail block length (3048 + 761*127 + 105 == 99800)
DMA_GRP = 8           # picard blocks per output DMA


# ---------------------------------------------------------------------------
# walrus workaround: CTRL-type instructions accept at most 1 sem wait, but the
# TileContext tail drain gets one wait per active proc. Split across drains.
def _patched_drain_and_barrier(self, tick_clock, wait_clock):
    drain_inst = self.nc.sync.drain()
    wait_clock.add_sem_waits(
        drain_inst.ins, ScopedClock({None: tick_clock.global_clock})
    )
    si = drain_inst.ins.sync_info
    waits = list(si.on_wait) if si is not None else []
    if len(waits) > 1:
        drain_inst.ins.sync_info = bass_rust.SyncInfo(
            on_wait=waits[:1], on_update=list(si.on_update)
        )
        for i in range(1, len(waits)):
            extra = self.nc.sync.drain()
            extra.ins.sync_info = bass_rust.SyncInfo(
                on_wait=waits[i : i + 1], on_update=[]
            )
    self.nc.all_engine_barrier()
    assert self.sems is not None
    popped = self.nc._tile_sem_poison_stack.pop()
    assert popped is self._sem_poison
    self.nc.clear_and_free_semaphores(list(self.sems.allocated().values()))
    self.nc.all_engine_barrier()


TileContext._drain_and_barrier = _patched_drain_and_barrier
# ---------------------------------------------------------------------------


def _split_excess_waits(nc, max_waits=1):
    """This walrus build accepts at most one sem wait per instruction; move
    excess waits onto single-wait NOPs inserted just before the owner."""
    n_split = 0
    for f in nc.m.functions:
        for b in f.blocks:
            insts = list(b.instructions)
            out = []
            changed = False
            for inst in insts:
                si = inst.sync_info
                waits = list(si.on_wait) if si is not None else []
                if len(waits) > max_waits:
                    changed = True
                    for i, w in enumerate(waits[max_waits:]):
                        nop = mybir.InstNoOp(
                            name=f"wsp_{inst.name}_{i}", ins=[], outs=[])
                        nop.engine = inst.engine
                        nop.sync_info = bass_rust.SyncInfo(
                            on_wait=[w], on_update=[])
                        out.append(nop)
                        n_split += 1
                    inst.sync_info = bass_rust.SyncInfo(
                        on_wait=waits[:max_waits], on_update=list(si.on_update))
                out.append(inst)
            if changed:
                b.instructions = out
    return n_split


def _drop_redundant_self_waits(nc, margin=64):
    """Drop semaphore waits that are provably satisfied at issue time:
    a wait on a sem that is (a) only ever incremented by compute
    instructions of the SAME engine as the waiter, and (b) whose wait
    value is at least `margin` increments behind the number of such
    increments emitted earlier in program order.  In-order engines
    guarantee those increments completed long before the waiter issues
    (margin covers exec-queue depth + write drain).  Fewer multi-wait
    instructions means fewer wait-split NOPs on the sequencers."""
    DMA_OPS = {"DMACopy", "TriggeredCopy", "DMATranspose", "CollectiveCompute"}
    n_drop = 0
    for f in nc.m.functions:
        # pass 1: sem id -> set of (engine, is_dma) updaters
        owners = {}
        for b in f.blocks:
            for inst in b.instructions:
                si = inst.sync_info
                if si is None:
                    continue
                for u in si.on_update:
                    key = u.id
                    owners.setdefault(key, set()).add(
                        (inst.engine, inst.opcode in DMA_OPS))
        solo = {
            sid: next(iter(s))[0]
            for sid, s in owners.items()
            if len(s) == 1 and not next(iter(s))[1]
        }
        # pass 2: walk in order, count increments, drop stale self-waits
        counts = {}
        for b in f.blocks:
            for inst in b.instructions:
                si = inst.sync_info
                if si is None:
                    continue
                waits = list(si.on_wait)
                kept = []
                for w in waits:
                    sid = w.id
                    if (sid in solo and solo[sid] == inst.engine
                            and w.wait_mode == "sem-ge-imm"
                            and w.wait_value is not None
                            and w.wait_value <= counts.get(sid, 0) - margin):
                        n_drop += 1
                        continue
                    kept.append(w)
                if len(kept) != len(waits):
                    inst.sync_info = bass_rust.SyncInfo(
                        on_wait=kept, on_update=list(si.on_update))
                for u in si.on_update:
                    if u.update_mode == "sem-inc":
                        counts[u.id] = counts.get(u.id, 0) + (
                            u.update_value or 1)
    return n_drop


def _pad_w5(w5):
    """(1,64,4,4) -> (4,4,64,32) with real weights in out-column 0."""
    t = np.zeros((4, 4, 64, 32), np.float32)
    t[:, :, :, 0:1] = w5.transpose(2, 3, 1, 0)
    return np.ascontiguousarray(t)


def _col_major_pad(v, ncols):
    """(n,) -> (128, ncols) with element m at [m % 128, m // 128], zero pad."""
    out = np.zeros(128 * ncols, np.float32)
    out[: v.shape[0]] = v
    return np.ascontiguousarray(out.reshape(ncols, 128).T)


def build_program(ser_groups=SER_G, n_blocks=None, n_m3=N_M3, tail=TAIL,
                  with_scan=True, timing=False, conv_lvl=9, ser_probe=0):
    if n_blocks is None:
        n_blocks = (T_FULL - tail) // KB - ser_groups
    assert ser_groups % 2 == 0, "sign-tile parity assumes even ser_groups"
    T = (ser_groups + n_blocks) * KB + tail
    nc = bass.Bass()

    # ---- inputs ----
    x_cols = nc.declare_dram_parameter("x_cols", [128, 16], f32, isOutput=False)
    win_t = nc.declare_dram_parameter("win_t", [2048, MROWS_A], f32, isOutput=False)
    bin_c = nc.declare_dram_parameter("bin_c", [128, 13], f32, isOutput=False)
    w1t = nc.declare_dram_parameter("w1t", [4, 4, 512, 512], f32, isOutput=False)
    w2t = nc.declare_dram_parameter("w2t", [4, 4, 512, 256], f32, isOutput=False)
    w3t = nc.declare_dram_parameter("w3t", [4, 4, 256, 128], f32, isOutput=False)
    w4t = nc.declare_dram_parameter("w4t", [4, 4, 128, 64], f32, isOutput=False)
    w5t = nc.declare_dram_parameter("w5t", [4, 4, 64, 32], f32, isOutput=False)
    g_all = nc.declare_dram_parameter("g_all", [128, 8], f32, isOutput=False)
    be_all = nc.declare_dram_parameter("be_all", [128, 8], f32, isOutput=False)
    wd2_t = nc.declare_dram_parameter("wd2_t", [6400, MROWS_C], f32, isOutput=False)
    bd2_c = nc.declare_dram_parameter("bd2_c", [128, 5], f32, isOutput=False)
    s0_in = nc.declare_dram_parameter("s0", [N, 1], f32, isOutput=False)
    ident_in = nc.declare_dram_parameter("ident", [128, 128], f32, isOutput=False)
    mtri_in = nc.declare_dram_parameter("mtri", [128, 128], f32, isOutput=False)
    ones_in = nc.declare_dram_parameter("ones", [128, 128], f32, isOutput=False)
    sgne_in = nc.declare_dram_parameter("sgne", [KB, N], f32, isOutput=False)
    sgno_in = nc.declare_dram_parameter("sgno", [KB, N], f32, isOutput=False)
    if with_scan:
        out_traj = nc.declare_dram_parameter("out", [T, N], f32, isOutput=True)
    else:
        w_out = nc.declare_dram_parameter("w_out", [N, N], f32, isOutput=True)

    # ---- internal DRAM ----
    h_shard = nc.dram_tensor("h_shard", [MROWS_A], f32)
    h_full = nc.dram_tensor("h_full", [NCORES * MROWS_A], f32, addr_space="Shared")
    c_scr = nc.dram_tensor("c_scr", [32, 6400], f32)
    wd_shard = nc.dram_tensor("wd_shard", [MROWS_C], f32)
    w_full = nc.dram_tensor("w_full", [NCORES * MROWS_C], f32, addr_space="Shared")

    with TileContext(nc) as tc:
        # ================= Phase A: h = W_in @ x + b_in (sharded) ==========
        with (
            tc.tile_pool(name="a_const", bufs=1) as acp,
            tc.tile_pool(name="a_slab", bufs=2) as asp,
            tc.tile_pool(name="a_ps", bufs=1, space="PSUM") as aps,
        ):
            xc = acp.tile([128, 16], f32)
            nc.sync.dma_start(out=xc[:, :], in_=x_cols[:, :])
            bc = acp.tile([128, 13], f32)
            nc.sync.dma_start(out=bc[:, :], in_=bin_c[:, :])
            hc = acp.tile([128, 13], f32)
            for jlo, jhi in ((0, 8), (8, 13)):
                ptiles = {}
                for j in range(jlo, jhi):
                    pt = aps.tile([128, 1], f32, tag=f"hps{j - jlo}", name=f"hps{j}")
                    ptiles[j] = pt
                for k in range(16):
                    gw = min(128 * jhi, MROWS_A) - 128 * jlo
                    slab = asp.tile([128, 1024], f32, tag="aslab")
                    nc.sync.dma_start(
                        out=slab[:, :gw],
                        in_=win_t[128 * k : 128 * (k + 1),
                                  128 * jlo : 128 * jlo + gw])
                    for j in range(jlo, jhi):
                        cj = 128 if j < 12 else 64
                        jj = j - jlo
                        nc.tensor.matmul(
                            ptiles[j][:cj, :],
                            slab[:, 128 * jj : 128 * jj + cj],
                            xc[:, k : k + 1],
                            start=(k == 0),
                            stop=(k == 15),
                        )
                for j in range(jlo, jhi):
                    cj = 128 if j < 12 else 64
                    nc.vector.tensor_tensor(
                        out=hc[:cj, j : j + 1], in0=ptiles[j][:cj, :],
                        in1=bc[:cj, j : j + 1], op=OP.add)
            for j in range(13):
                cj = 128 if j < 12 else 64
                nc.sync.dma_start(
                    out=h_shard[128 * j : 128 * j + cj], in_=hc[:cj, j])
        if timing:
            # timing build (TimelineSim is single-core): local DMA stand-in;
            # the analytic collective cost is added by the caller.
            nc.sync.dma_start(out=h_full[0:MROWS_A], in_=h_shard[:])
        else:
            nc.gpsimd.collective_compute(
                "AllGather", OP.bypass, replica_groups=[list(range(NCORES))],
                ins=[h_shard[:]], outs=[h_full[:]])

        # ================= Phase B: conv stack (replicated) ================
        _lvl = conv_lvl  # conv-layer bisection gate (9 = all layers)
        h2d = h_full.rearrange("(c hw) -> c hw", hw=25)
        gsl = {1: (0, 4), 2: (4, 2), 3: (6, 1), 4: (7, 1)}  # (col offset, ncols)

        with (
            tc.tile_pool(name="bn_const", bufs=1) as bnp,
            tc.tile_pool(name="conv_ps", bufs=1, space="PSUM") as bps,
        ):
            g_sb = bnp.tile([128, 8], f32)
            nc.sync.dma_start(out=g_sb[:, :], in_=g_all[:, :])
            be_sb = bnp.tile([128, 8], f32)
            nc.sync.dma_start(out=be_sb[:, :], in_=be_all[:, :])

            def bn_relu(raw, hw, cch, lidx, j, out_ap):
                """BatchNorm(train) + ReLU from raw (cch,hw) into out_ap."""
                with tc.tile_pool(name=f"bn{lidx}_{j}", bufs=1) as p:
                    s1 = p.tile([cch, 1], f32, tag="s1")
                    nc.vector.tensor_reduce(s1[:, :], raw, axis=AX.X, op=OP.add)
                    mean = p.tile([cch, 1], f32, tag="mean")
                    nc.vector.tensor_scalar_mul(mean[:, :], s1[:, :], 1.0 / hw)
                    sq = p.tile([cch, hw], f32, tag="sq")
                    nc.vector.tensor_tensor(out=sq[:, :], in0=raw, in1=raw, op=OP.mult)
                    s2 = p.tile([cch, 1], f32, tag="s2")
                    nc.vector.tensor_reduce(s2[:, :], sq[:, :], axis=AX.X, op=OP.add)
                    ex2 = p.tile([cch, 1], f32, tag="ex2")
                    nc.vector.tensor_scalar_mul(ex2[:, :], s2[:, :], 1.0 / hw)
                    msq = p.tile([cch, 1], f32, tag="msq")
                    nc.vector.tensor_tensor(
                        out=msq[:, :], in0=mean[:, :], in1=mean[:, :], op=OP.mult)
                    var = p.tile([cch, 1], f32, tag="var")
                    nc.vector.tensor_tensor(
                        out=var[:, :], in0=ex2[:, :], in1=msq[:, :], op=OP.subtract)
                    vps = p.tile([cch, 1], f32, tag="vps")
                    nc.vector.tensor_scalar_add(vps[:, :], var[:, :], EPS)
                    sd = p.tile([cch, 1], f32, tag="sd")
                    nc.scalar.activation(sd[:, :], vps[:, :], AF.Sqrt)
                    rstd = p.tile([cch, 1], f32, tag="rstd")
                    nc.vector.reciprocal(rstd[:, :], sd[:, :])
                    co, _ = gsl[lidx]
                    scale = p.tile([cch, 1], f32, tag="scale")
                    nc.vector.tensor_tensor(
                        out=scale[:, :], in0=g_sb[:cch, co + j : co + j + 1],
                        in1=rstd[:, :], op=OP.mult)
                    t1 = p.tile([cch, 1], f32, tag="t1")
                    nc.vector.tensor_tensor(
                        out=t1[:, :], in0=mean[:, :], in1=scale[:, :], op=OP.mult)
                    bia = p.tile([cch, 1], f32, tag="bia")
                    nc.vector.tensor_tensor(
                        out=bia[:, :], in0=be_sb[:cch, co + j : co + j + 1],
                        in1=t1[:, :], op=OP.subtract)
                    nc.scalar.activation(
                        out_ap, raw, AF.Relu, bias=bia[:, :], scale=scale[:, :])

            # ---- L1: up2(h:512x5x5)->512x10x10 conv 512->512 ----
            with (
                tc.tile_pool(name="l1_in", bufs=1) as l1i,
                tc.tile_pool(name="l1_w", bufs=2) as l1w,
                tc.tile_pool(name="l1_out", bufs=1) as l1o,
            ):
                pads1 = []
                for j in range(4):
                    hm = l1i.tile([128, 25], f32, tag=f"hm{j}")
                    nc.sync.dma_start(out=hm[:, :], in_=h2d[128 * j : 128 * (j + 1), :])
                    pad = l1i.tile([128, 13 * 13], f32, tag=f"pad1_{j}")
                    nc.vector.memset(pad[:, :], 0.0)
                    pv = pad[:, :].rearrange("c (h w) -> c h w", h=13)
                    hv = hm[:, :].rearrange("c (h w) -> c h w", h=5)
                    for a in range(2):
                        for b in range(2):
                            nc.vector.tensor_copy(
                                pv[:, a + 1 : a + 11 : 2, b + 1 : b + 11 : 2], hv[:, :, :])
                    pads1.append(pad)
                ps1s = []
                for jo in range(4):
                    p1 = bps.tile([128, 100], f32, tag=f"l1ps{jo}", name=f"l1ps{jo}")
                    ps1s.append(p1)
                nmm = 0
                for ji in range(4):
                    for dy in range(4):
                        for dx in range(4):
                            slab = l1w.tile([128, 512], f32, tag="w1slab")
                            nc.sync.dma_start(
                                out=slab[:, :],
                                in_=w1t[dy, dx, 128 * ji : 128 * (ji + 1), :])
                            rhs = pads1[ji][:, :].rearrange(
                                "c (h w) -> c h w", h=13)[:, dy : dy + 10, dx : dx + 10]
                            for jo in range(4):
                                nc.tensor.matmul(
                                    ps1s[jo][:, :],
                                    slab[:, 128 * jo : 128 * (jo + 1)], rhs,
                                    start=(nmm == 0), stop=(nmm == 63))
                            nmm += 1
                pads2 = []
                for jo in range(4):
                    raw = l1o.tile([128, 100], f32, tag=f"raw1_{jo}")
                    nc.vector.tensor_copy(raw[:, :], ps1s[jo][:, :])
                    relu = l1o.tile([128, 100], f32, tag=f"relu1_{jo}")
                    bn_relu(raw[:, :], 100, 128, 1, jo, relu[:, :])
                    pad = l1o.tile([128, 23 * 23], f32, tag=f"pad2_{jo}")
                    nc.vector.memset(pad[:, :], 0.0)
                    pv = pad[:, :].rearrange("c (h w) -> c h w", h=23)
                    rv = relu[:, :].rearrange("c (h w) -> c h w", h=10)
                    for a in range(2):
                        for b in range(2):
                            nc.vector.tensor_copy(
                                pv[:, a + 1 : a + 21 : 2, b + 1 : b + 21 : 2], rv[:, :, :])
                    pads2.append(pad)

                if _lvl >= 2:
                  # ---- L2: 512x20x20 conv 512->256 ----
                  with (
                      tc.tile_pool(name="l2_w", bufs=2) as l2w,
                      tc.tile_pool(name="l2_out", bufs=1) as l2o,
                  ):
                      psA = bps.tile([128, 400], f32, tag="cpsA")
                      psB = bps.tile([128, 400], f32, tag="cpsB")
                      nmm = 0
                      for ji in range(4):
                          for dy in range(4):
                              for dx in range(4):
                                  slab = l2w.tile([128, 256], f32, tag="w2slab")
                                  nc.sync.dma_start(
                                      out=slab[:, :],
                                      in_=w2t[dy, dx, 128 * ji : 128 * (ji + 1), :])
                                  rhs = pads2[ji][:, :].rearrange(
                                      "c (h w) -> c h w", h=23)[:, dy : dy + 20, dx : dx + 20]
                                  nc.tensor.matmul(
                                      psA[:, :], slab[:, 0:128], rhs,
                                      start=(nmm == 0), stop=(nmm == 63))
                                  nc.tensor.matmul(
                                      psB[:, :], slab[:, 128:256], rhs,
                                      start=(nmm == 0), stop=(nmm == 63))
                                  nmm += 1
                      pads3 = []
                      for jo, ps in enumerate((psA, psB)):
                          raw = l2o.tile([128, 400], f32, tag=f"raw2_{jo}")
                          nc.vector.tensor_copy(raw[:, :], ps[:, :])
                          relu = l2o.tile([128, 400], f32, tag=f"relu2_{jo}")
                          bn_relu(raw[:, :], 400, 128, 2, jo, relu[:, :])
                          pad = l2o.tile([128, 43 * 43], f32, tag=f"pad3_{jo}")
                          nc.vector.memset(pad[:, :], 0.0)
                          pv = pad[:, :].rearrange("c (h w) -> c h w", h=43)
                          rv = relu[:, :].rearrange("c (h w) -> c h w", h=20)
                          for a in range(2):
                              for b in range(2):
                                  nc.vector.tensor_copy(
                                      pv[:, a + 1 : a + 41 : 2, b + 1 : b + 41 : 2],
                                      rv[:, :, :])
                          pads3.append(pad)

                      if _lvl >= 3:
                        # ---- L3: 256x40x40 conv 256->128 ----
                        with (
                            tc.tile_pool(name="l3_w", bufs=1) as l3w,
                            tc.tile_pool(name="l3_out", bufs=1) as l3o,
                        ):
                            wsl3 = l3w.tile([128, 32 * 128], f32)
                            for ji in range(2):
                                for dy in range(4):
                                    for dx in range(4):
                                        si = (ji * 16 + dy * 4 + dx) * 128
                                        nc.sync.dma_start(
                                            out=wsl3[:, si : si + 128],
                                            in_=w3t[dy, dx, 128 * ji : 128 * (ji + 1), :])
                            raw3 = l3o.tile([128, 1600], f32)
                            for st in range(4):
                                ps = bps.tile([128, 400], f32, tag="cps", bufs=2)
                                nmm = 0
                                for ji in range(2):
                                    for dy in range(4):
                                        for dx in range(4):
                                            si = (ji * 16 + dy * 4 + dx) * 128
                                            rhs = pads3[ji][:, :].rearrange(
                                                "c (h w) -> c h w", h=43)[
                                                :, st * 10 + dy : st * 10 + dy + 10,
                                                dx : dx + 40]
                                            nc.tensor.matmul(
                                                ps[:, :], wsl3[:, si : si + 128], rhs,
                                                start=(nmm == 0), stop=(nmm == 31))
                                            nmm += 1
                                nc.vector.tensor_copy(
                                    raw3[:, 400 * st : 400 * (st + 1)], ps[:, :])
                            relu3 = l3o.tile([128, 1600], f32)
                            bn_relu(raw3[:, :], 1600, 128, 3, 0, relu3[:, :])
                            pad4 = l3o.tile([128, 83 * 83], f32)
                            nc.vector.memset(pad4[:, :], 0.0)
                            pv = pad4[:, :].rearrange("c (h w) -> c h w", h=83)
                            rv = relu3[:, :].rearrange("c (h w) -> c h w", h=40)
                            for a in range(2):
                                for b in range(2):
                                    nc.vector.tensor_copy(
                                        pv[:, a + 1 : a + 81 : 2, b + 1 : b + 81 : 2],
                                        rv[:, :, :])

                            if _lvl >= 4:
                              # ---- L4: 128x80x80 conv 128->64 ----
                              with (
                                  tc.tile_pool(name="l4_w", bufs=1) as l4w,
                                  tc.tile_pool(name="l4_out", bufs=1) as l4o,
                              ):
                                  wsl4 = l4w.tile([128, 16 * 64], f32)
                                  for dy in range(4):
                                      for dx in range(4):
                                          si = (dy * 4 + dx) * 64
                                          nc.sync.dma_start(
                                              out=wsl4[:, si : si + 64],
                                              in_=w4t[dy, dx, :, :])
                                  raw4 = l4o.tile([64, 6400], f32)
                                  for st in range(16):
                                      ps = bps.tile([64, 400], f32, tag="cps", bufs=2)
                                      nmm = 0
                                      for dy in range(4):
                                          for dx in range(4):
                                              si = (dy * 4 + dx) * 64
                                              rhs = pad4[:, :].rearrange(
                                                  "c (h w) -> c h w", h=83)[
                                                  :, st * 5 + dy : st * 5 + dy + 5,
                                                  dx : dx + 80]
                                              nc.tensor.matmul(
                                                  ps[:, :], wsl4[:, si : si + 64], rhs,
                                                  start=(nmm == 0), stop=(nmm == 15))
                                              nmm += 1
                                      nc.vector.tensor_copy(
                                          raw4[:, 400 * st : 400 * (st + 1)], ps[:, :])
                                  pad5 = l4o.tile([64, 83 * 83], f32)
                                  nc.vector.memset(pad5[:, :], 0.0)
                                  pv5 = pad5[:, :].rearrange("c (h w) -> c h w", h=83)[
                                      :, 1:81, 1:81]
                                  bn_relu(raw4[:, :], 6400, 64, 4, 0, pv5)

                                  if _lvl >= 5:
                                    # ---- L5: 64x80x80 conv 64->1 + tanh -> c ----
                                    with (
                                        tc.tile_pool(name="l5_w", bufs=1) as l5w,
                                        tc.tile_pool(name="l5_out", bufs=1) as l5o,
                                    ):
                                        wsl5 = l5w.tile([64, 16 * 32], f32)
                                        for dy in range(4):
                                            for dx in range(4):
                                                _p5 = (dy * 4 + dx) * 32
                                                nc.sync.dma_start(
                                                    out=wsl5[:, _p5 : _p5 + 32],
                                                    in_=w5t[dy, dx, :, :])
                                        for st in range(16):
                                            ps = bps.tile([32, 400], f32, tag="cps", bufs=2)
                                            nmm = 0
                                            for dy in range(4):
                                                for dx in range(4):
                                                    rhs = pad5[:, :].rearrange(
                                                        "c (h w) -> c h w", h=83)[
                                                        :, st * 5 + dy : st * 5 + dy + 5,
                                                        dx : dx + 80]
                                                    _p5 = (dy * 4 + dx) * 32
                                                    nc.tensor.matmul(
                                                        ps[:, :],
                                                        wsl5[:, _p5 : _p5 + 32],
                                                        rhs,
                                                        start=(nmm == 0), stop=(nmm == 15))
                                                    nmm += 1
                                            c32 = l5o.tile([32, 400], f32, tag="c32", name=f"c32_{st}")
                                            nc.scalar.activation(c32[:, :], ps[:, :], AF.Tanh)
                                            nc.sync.dma_start(
                                                out=c_scr[:, 400 * st : 400 * (st + 1)], in_=c32[:, :])

        # ================= Phase C: w = W_d2 @ c + b_d2 (sharded) ==========
        _skip_c = False
        if not _skip_c:
          with (
              tc.tile_pool(name="c_const", bufs=1) as ccp,
              tc.tile_pool(name="c_slab", bufs=2) as csp,
              tc.tile_pool(name="c_ps", bufs=1, space="PSUM") as cps,
          ):
              c_cols = ccp.tile([128, 50], f32)
              nc.sync.dma_start(
                  out=c_cols[:, :], in_=c_scr[0, :].rearrange("(f p) -> p f", p=128))
              bdc = ccp.tile([128, 5], f32)
              nc.sync.dma_start(out=bdc[:, :], in_=bd2_c[:, :])
              wtiles = {}
              for j in range(5):
                  wt_ps = cps.tile([128, 1], f32, tag=f"wps{j}", name=f"wps{j}")
                  wtiles[j] = wt_ps
              for k in range(50):
                  slab = csp.tile([128, MROWS_C], f32, tag="cslab")
                  nc.sync.dma_start(
                      out=slab[:, :], in_=wd2_t[128 * k : 128 * (k + 1), :])
                  for j in range(5):
                      cj = 128 if j < 4 else 84
                      nc.tensor.matmul(
                          wtiles[j][:cj, :], slab[:, 128 * j : 128 * j + cj],
                          c_cols[:, k : k + 1], start=(k == 0), stop=(k == 49))
              wdc = ccp.tile([128, 5], f32)
              for j in range(5):
                  cj = 128 if j < 4 else 84
                  nc.vector.tensor_tensor(
                      out=wdc[:cj, j : j + 1], in0=wtiles[j][:cj, :],
                      in1=bdc[:cj, j : j + 1], op=OP.add)
              for j in range(5):
                  cj = 128 if j < 4 else 84
                  nc.sync.dma_start(
                      out=wd_shard[128 * j : 128 * j + cj], in_=wdc[:cj, j])
        if not _skip_c:
            if timing:
                nc.sync.dma_start(out=w_full[0:MROWS_C], in_=wd_shard[:])
            else:
                nc.gpsimd.collective_compute(
                    "AllGather", OP.bypass, replica_groups=[list(range(NCORES))],
                    ins=[wd_shard[:]], outs=[w_full[:]])

        if not with_scan:
            with tc.tile_pool(name="wout", bufs=1) as wop:
                w_sb0 = wop.tile([N, N], f32)
                nc.sync.dma_start(
                    out=w_sb0[:, :],
                    in_=w_full[0 : N * N].rearrange("(j i) -> j i", i=N))
                nc.sync.dma_start(out=w_out[:, :], in_=w_sb0[:, :])

        # ================= Phase D: spiking scan =========================
        if with_scan:
          ms = [3] * min(n_m3, n_blocks) + [2] * max(0, n_blocks - n_m3)
          kbs = [KB] * n_blocks
          if tail:
              ms.append(3)
              kbs.append(tail)
          nbt = len(ms)           # picard blocks incl tail
          ser_steps = ser_groups * KB
          with (
              tc.tile_pool(name="d_const", bufs=1) as dcp,
              tc.tile_pool(name="d_sb", bufs=2) as dsb,
          ):
            w_sb = dcp.tile([N, N], f32)
            nc.sync.dma_start(
                out=w_sb[:, :],
                in_=w_full[0 : N * N].rearrange("(j i) -> j i", i=N))
            wneg = dcp.tile([N, N], f32)
            nc.vector.tensor_scalar_mul(wneg[:, :], w_sb[:, :], -1.0)
            mtri = dcp.tile([128, 128], f32)
            nc.sync.dma_start(out=mtri[:, :], in_=mtri_in[:, :])
            ident = dcp.tile([128, 128], f32)
            nc.sync.dma_start(out=ident[:, :], in_=ident_in[:, :])
            onesm = dcp.tile([128, 128], f32)
            nc.sync.dma_start(out=onesm[:, :], in_=ones_in[:, :])
            sgne = dcp.tile([KB, N], f32)
            nc.sync.dma_start(out=sgne[:, :], in_=sgne_in[:, :])
            sgno = dcp.tile([KB, N], f32)
            nc.sync.dma_start(out=sgno[:, :], in_=sgno_in[:, :])
            s0c = dcp.tile([N, 1], f32)
            nc.sync.dma_start(out=s0c[:, :], in_=s0_in[:, :])

            # picard block state tiles (allocated up front so the serial
            # handoff can fill block 0 before the serial PSUM pool closes)
            ms = list(ms)
            ub_t = [None] * nbt
            s0_t = [None] * nbt          # [1,N] start-state rows (partition 0)
            st_t = [None] * nbt
            ub_t[0] = dsb.tile([KB, N], f32, tag="ub", bufs=3, name="ub0")
            s0_t[0] = dsb.tile([1, N], f32, tag="s0row", bufs=3, name="s0r0")

            # ---------- serial phase: 2-op steps, t-major output ----------
            # ub_ser: cols 0..126 = u' of the group, col 127 = group start s'
            serial_psum = tc.tile_pool(name="d_ps_ser", bufs=2, space="PSUM")
            dps = serial_psum.__enter__()
            ub_ser = dcp.tile([N, 128], f32)
            nc.vector.tensor_copy(ub_ser[:, 127:128], s0c[:, :])
            ybank = dps.tile([N, 1], f32, tag="ybank", name="ybank", bufs=1)
            nc.tensor.matmul(
                ybank[:, :], w_sb[:, :], s0c[:, :], start=True, stop=True)

            ser_dma_grp = 4       # serial groups per output DMA
            obufS = None
            last_ubT = None       # SBUF [128,N] of the last serial group
            last_stS = None       # PSUM [128,N] finish of the last group
            for g in range(ser_groups):
                for k in range(KB):
                    nc.scalar.activation(
                        ub_ser[:, k : k + 1], ybank[:, :], AF.Tanh)
                    nc.tensor.matmul(
                        ybank[:, :], wneg[:, :], ub_ser[:, k : k + 1],
                        start=False, stop=True, skip_group_check=True)
                if ser_probe == 1 and g < ser_groups - 1:
                    continue
                # group recon: transpose -> prefix -> sign -> batched DMA
                ubT_ps = dps.tile([128, N], f32, tag="ubTps")
                nc.tensor.transpose(ubT_ps[:, :], ub_ser[:, :], ident[:N, :N])
                ubT_sb = dsb.tile([128, N], f32, tag="ubTsb")
                nc.vector.tensor_copy(ubT_sb[:, :], ubT_ps[:, :])
                stS_ps = dps.tile([KB, N], f32, tag="stSps")
                nc.tensor.matmul(
                    stS_ps[:, :], mtri[:, 1 : KB + 1], ubT_sb[:, :],
                    start=True, stop=True)
                gi = g % ser_dma_grp
                if gi == 0 or obufS is None:
                    obufS = dsb.tile([KB, ser_dma_grp * N], f32, tag="obufS")
                sg = sgne if g % 2 == 0 else sgno
                nc.vector.tensor_tensor(
                    out=obufS[:, gi * N : (gi + 1) * N],
                    in0=stS_ps[:, :], in1=sg[:, :], op=OP.mult)
                ngrp = min(ser_dma_grp, ser_groups - (g - gi))
                if gi == ngrp - 1:
                    r0 = (g - gi) * KB
                    nc.sync.dma_start(
                        out=out_traj[r0 : r0 + ngrp * KB, :].rearrange(
                            "(b p) n -> p b n", b=ngrp),
                        in_=obufS[:, : ngrp * N].rearrange(
                            "p (b n) -> p b n", b=ngrp))
                # next start column via single-col prefix matmul
                scol_ps = dps.tile([N, 1], f32, tag="scolps")
                nc.tensor.matmul(
                    scol_ps[:, :], ubT_sb[:, :], mtri[:, 127:128],
                    start=True, stop=True)
                if g < ser_groups - 1:
                    nc.vector.tensor_copy(ub_ser[:, 127:128], scol_ps[:, :])
                else:
                    last_ubT = ubT_sb
                    last_scol = scol_ps

            # exact serial -> block-0 handoff (inside the serial PSUM
            # scope: last_scol is a PSUM tile). The end-state column is
            # transposed into a partition-0 row.
            nc.gpsimd.tensor_copy(ub_t[0][:, :], last_ubT[0:KB, :])
            scol_sb = dsb.tile([N, 1], f32, tag="scolsb")
            nc.vector.tensor_copy(scol_sb[:, :], last_scol[:, :])
            s0T_ps = dps.tile([1, N], f32, tag="s0Tps", bufs=1)
            nc.tensor.transpose(s0T_ps[:, :], scol_sb[:, :], ident[:N, :N])
            nc.vector.tensor_copy(s0_t[0][:, :], s0T_ps[:, :])
            serial_psum.__exit__(None, None, None)

            # ---------- pipelined blocked-Picard phase (LIT) ----------
            # Per block b: seeds + preview start from block b-1's U^{(M-1)}
            # (sum via e127 matmul); only the last iteration waits for the
            # true (converged) start state of the block.
            picard_psum = tc.tile_pool(name="d_ps_pic", bufs=2, space="PSUM")
            dps = picard_psum.__enter__()
            obuf = None
            obuf_base = 0

            pend_sp = [None] * nbt    # hoisted u-part prefix (final iter)

            def emit_mm1a(b):
                """u-part prefix matmul of block b's FINAL iteration, hoisted
                off the critical chain (only needs tanh(b, M-1))."""
                kb = kbs[b]
                sp_ps = dps.tile([N, 128], f32, tag="spps")
                nc.tensor.matmul(
                    sp_ps[:, :kb], ub_t[b][:, :], mtri[0:KB, :kb],
                    start=True, stop=False, skip_group_check=True)
                pend_sp[b] = sp_ps

            def emit_iter(b, final=False):
                """One Picard iteration of block b (mm1 pair,copy,mm2,tanh).
                S' cols = u-row prefix (strict-lower mtri) + s0 broadcast."""
                kb = kbs[b]
                if final:
                    sp_ps = pend_sp[b]
                else:
                    sp_ps = dps.tile([N, 128], f32, tag="spps")
                    nc.tensor.matmul(
                        sp_ps[:, :kb], ub_t[b][:, :], mtri[0:KB, :kb],
                        start=True, stop=False, skip_group_check=True)
                nc.tensor.matmul(
                    sp_ps[:, :kb], s0_t[b][:, :], onesm[0:1, :kb],
                    start=False, stop=True, skip_group_check=True)
                sp_sb = dsb.tile([N, KB], f32, tag="spsb")
                nc.vector.tensor_copy(sp_sb[:, :kb], sp_ps[:, :kb])
                y_ps = dps.tile([KB, N], f32, tag="yps")
                nc.tensor.matmul(
                    y_ps[:kb, :], sp_sb[:, :kb], w_sb[:, :],
                    start=True, stop=True)
                nc.scalar.activation(ub_t[b][0:kb, :], y_ps[:kb, :], AF.Tanh)

            for s in range(nbt + 1):
                # --- pre-ops of block s (reads block s-1's U^{(M-1)}) ---
                if 0 < s < nbt:
                    ub_t[s] = dsb.tile([KB, N], f32, tag="ub", bufs=3,
                                       name=f"ub{s}")
                    s0_t[s] = dsb.tile([1, N], f32, tag="s0row", bufs=3,
                                       name=f"s0r{s}")
                    if True:
                        sum_ps = dps.tile([1, N], f32, tag="sumps")
                        nc.tensor.matmul(
                            sum_ps[:, :], onesm[0:KB, 0:1], ub_t[s - 1][:, :],
                            start=True, stop=True)
                        nc.vector.tensor_tensor(
                            out=s0_t[s][:, :], in0=s0_t[s - 1][:, :],
                            in1=sum_ps[:, :], op=OP.subtract)
                        nc.gpsimd.tensor_copy(
                            ub_t[s][:, :], ub_t[s - 1][:, :])
                # --- final iteration of block s-1 (u-part hoisted) ---
                if s > 0:
                    emit_iter(s - 1, final=True)
                # --- iterations 1..M-1 of block s (preview start) ---
                if s < nbt:
                    for m in range(1, ms[s]):
                        emit_iter(s)
                # --- true-start handoff for block s (after tanh(s-1, M),
                #     before the final iteration emitted next stage) ---
                if 0 < s < nbt:
                    sum2_ps = dps.tile([1, N], f32, tag="sumps",
                                       name=f"sum2_{s}")
                    nc.tensor.matmul(
                        sum2_ps[:, :], onesm[0:KB, 0:1], ub_t[s - 1][:, :],
                        start=True, stop=True)
                    nc.vector.tensor_tensor(
                        out=s0_t[s][:, :], in0=s0_t[s - 1][:, :],
                        in1=sum2_ps[:, :], op=OP.subtract)
                # --- hoist the final iteration's u-part prefix (after T1 so
                #     it cannot head-of-line block the true-start sum) ---
                if s < nbt:
                    emit_mm1a(s)
                # --- finish + output of block s-1 ---
                if s > 0:
                    b = s - 1
                    kb = kbs[b]
                    st_t[b] = dps.tile([KB, N], f32, tag="stps",
                                       name=f"st{b}")
                    nc.tensor.matmul(
                        st_t[b][:kb, :], mtri[0:KB, 1 : kb + 1], ub_t[b][:, :],
                        start=True, stop=False)
                    nc.tensor.matmul(
                        st_t[b][:kb, :], onesm[0:1, 0:kb], s0_t[b][:, :],
                        start=False, stop=True)
                    if b < n_blocks:
                        gi = b % DMA_GRP
                        if gi == 0:
                            obuf = dsb.tile([KB, DMA_GRP * N], f32,
                                            tag="obuf")
                            obuf_base = b
                        sg = sgne if b % 2 == 0 else sgno
                        nc.vector.tensor_tensor(
                            out=obuf[:, gi * N : (gi + 1) * N],
                            in0=st_t[b][:, :], in1=sg[:, :],
                            op=OP.mult)
                        ngrp = min(DMA_GRP, n_blocks - obuf_base)
                        if gi == ngrp - 1:
                            r0 = ser_steps + obuf_base * KB
                            nc.sync.dma_start(
                                out=out_traj[
                                    r0 : r0 + ngrp * KB, :].rearrange(
                                    "(b p) n -> p b n", b=ngrp),
                                in_=obuf[:, : ngrp * N].rearrange(
                                    "p (b n) -> p b n", b=ngrp))
                    else:
                        # tail block: sign pattern continues the parity of
                        # block index b (tail rows start at an odd offset)
                        sg = sgne if b % 2 == 0 else sgno
                        otail = dsb.tile([KB, N], f32, tag="otail")
                        nc.vector.tensor_tensor(
                            out=otail[:kb, :], in0=st_t[b][:kb, :],
                            in1=sg[:kb, :], op=OP.mult)
                        nc.sync.dma_start(
                            out=out_traj[T - kb : T, :], in_=otail[:kb, :])
            picard_psum.__exit__(None, None, None)

    return nc


def _marshal_inputs(inputs):
    """Build the 8 per-core input maps from the full problem inputs."""
    x = np.asarray(inputs["x"], np.float32).reshape(2048)
    win = np.asarray(inputs["W_in"], np.float32)
    b_in = np.asarray(inputs["b_in"], np.float32)
    wd2 = np.asarray(inputs["W_d2"], np.float32)
    bd2 = np.asarray(inputs["b_d2"], np.float32)
    sp = np.asarray(inputs["start_part"], np.float32)

    x_cols = np.ascontiguousarray(x.reshape(16, 128).T)
    g_all = np.zeros((128, 8), np.float32)
    be_all = np.zeros((128, 8), np.float32)
    g_all[:, 0:4] = _col_major_pad(np.asarray(inputs["g1"], np.float32), 4)
    g_all[:, 4:6] = _col_major_pad(np.asarray(inputs["g2"], np.float32), 2)
    g_all[:, 6:7] = _col_major_pad(np.asarray(inputs["g3"], np.float32), 1)
    g_all[:, 7:8] = _col_major_pad(np.asarray(inputs["g4"], np.float32), 1)
    be_all[:, 0:4] = _col_major_pad(np.asarray(inputs["be1"], np.float32), 4)
    be_all[:, 4:6] = _col_major_pad(np.asarray(inputs["be2"], np.float32), 2)
    be_all[:, 6:7] = _col_major_pad(np.asarray(inputs["be3"], np.float32), 1)
    be_all[:, 7:8] = _col_major_pad(np.asarray(inputs["be4"], np.float32), 1)
    wts = {
        "w1t": np.ascontiguousarray(
            np.asarray(inputs["w1"], np.float32).transpose(2, 3, 1, 0)),
        "w2t": np.ascontiguousarray(
            np.asarray(inputs["w2"], np.float32).transpose(2, 3, 1, 0)),
        "w3t": np.ascontiguousarray(
            np.asarray(inputs["w3"], np.float32).transpose(2, 3, 1, 0)),
        "w4t": np.ascontiguousarray(
            np.asarray(inputs["w4"], np.float32).transpose(2, 3, 1, 0)),
        "w5t": _pad_w5(np.asarray(inputs["w5"], np.float32)),
    }
    s0 = np.ascontiguousarray(sp[-1].reshape(N, 1))
    ident = np.eye(128, dtype=np.float32)
    # prefix matrix: S'[i,t] = sum_k ubT[k,i]*mtri[k,t]; strict-lower -1s
    # for the u' rows, +1 base row (127) for the s'0 term.
    mtri = np.zeros((128, 128), np.float32)
    for k in range(127):
        mtri[k, k + 1 :] = -1.0
    mtri[127, :] = 1.0
    # all-ones helper (column sums / base-row broadcasts via matmul)
    onesm = np.ones((128, 128), np.float32)
    # unpriming signs by output row parity: out[t] = (-1)^(t+1) s'_{t+1};
    # within a group starting at even global t, row j gets (-1)^(j+1).
    sgne = np.tile(
        np.where(np.arange(KB) % 2 == 0, -1.0, 1.0
                 ).astype(np.float32)[:, None], (1, N))
    sgno = -sgne

    wd2_pad = np.zeros((NCORES * MROWS_C, 6400), np.float32)
    wd2_pad[: wd2.shape[0]] = wd2
    bd2_pad = np.zeros(NCORES * MROWS_C, np.float32)
    bd2_pad[: bd2.shape[0]] = bd2

    in_maps = []
    for c in range(NCORES):
        m = {
            "x_cols": x_cols,
            "win_t": np.ascontiguousarray(
                win[MROWS_A * c : MROWS_A * (c + 1)].T),
            "bin_c": _col_major_pad(b_in[MROWS_A * c : MROWS_A * (c + 1)], 13),
            "g_all": g_all,
            "be_all": be_all,
            "wd2_t": np.ascontiguousarray(
                wd2_pad[MROWS_C * c : MROWS_C * (c + 1)].T),
            "bd2_c": _col_major_pad(bd2_pad[MROWS_C * c : MROWS_C * (c + 1)], 5),
            "s0": s0,
            "ident": ident,
            "mtri": mtri,
            "ones": onesm,
            "sgne": sgne,
            "sgno": sgno,
        }
        m.update(wts)
        in_maps.append(m)
    return in_maps


LAST_EXEC_NS = None


def kernel(**inputs) -> np.ndarray:
    global LAST_EXEC_NS
    import os

    trace = bool(os.environ.get("KERNEL_TRACE"))
    nc = build_program()
    _drop_redundant_self_waits(nc)
    _split_excess_waits(nc)
    in_maps = _marshal_inputs(inputs)
    res = run_bass_kernel_spmd(nc, in_maps, list(range(NCORES)), trace=trace)
    if res.exec_time_ns is not None:
        LAST_EXEC_NS = res.exec_time_ns
    out = np.asarray(res.results[0]["out"], np.float32)
    return out.reshape(1, T_FULL, N)


def _host_device_sim(w, s_init, ser_groups=SER_G, n_blocks=None, n_m3=N_M3,
                     tail=TAIL):
    """Numpy mirror of the device schedule (pipelined LIT semantics)."""
    if n_blocks is None:
        n_blocks = (T_FULL - tail) // KB - ser_groups
    ser_steps = ser_groups * KB
    T = ser_steps + n_blocks * KB + tail
    ms = [3] * min(n_m3, n_blocks) + [2] * max(0, n_blocks - n_m3)
    kbs = [KB] * n_blocks
    if tail:
        ms.append(3)
        kbs.append(tail)
    out_p = np.empty((T, N), np.float32)
    yp = (s_init @ w).astype(np.float32)
    sp = s_init.copy()
    ubh = np.zeros((KB, N), np.float32)
    for t in range(ser_steps):
        up = np.tanh(yp).astype(np.float32)
        ubh[t % KB] = up
        yp = (yp - (up @ w).astype(np.float32)).astype(np.float32)
        sp = (sp - up).astype(np.float32)
        out_p[t] = sp

    def prefix(s0, U, Kb):
        S = np.empty((Kb + 1, N), np.float32)
        S[0] = s0
        S[1:] = s0 - np.cumsum(U[:Kb], axis=0, dtype=np.float32)
        return S

    true_prev = out_p[ser_steps - 1].copy()   # true start of block 0 (exact)
    seeds = ubh.copy()                        # U^{(M-1)} of "block -1"
    t = ser_steps
    for b in range(len(ms)):
        M, kb = ms[b], kbs[b]
        if b == 0:
            start_all = true_prev             # exact for every iteration
            true_b = true_prev
        else:
            preview = (true_prev - seeds.sum(axis=0,
                                             dtype=np.float32)).astype(
                np.float32)
            start_all = preview
            true_b = None                     # filled after U^{(M)} known
        U = seeds.copy()
        for m in range(1, M):                 # preview iterations
            S = prefix(start_all, U, kb)
            Y = (S[:kb] @ w).astype(np.float32)
            U = np.tanh(Y).astype(np.float32)
        if b == 0:
            true_b = true_prev
        else:
            # sum2 over prev block's converged U^{(M)}
            true_b = (true_prev - prev_conv.sum(axis=0,
                                                dtype=np.float32)).astype(
                np.float32)
        seeds_next = U.copy()                 # U^{(M-1)} of this block
        S = prefix(true_b, U, kb)             # final iteration (true start)
        Y = (S[:kb] @ w).astype(np.float32)
        U = np.tanh(Y).astype(np.float32)
        St = prefix(true_b, U, kb)            # finish
        out_p[t : t + kb] = St[1 : kb + 1]
        prev_conv = U
        seeds = seeds_next
        true_prev = true_b
        t += kb
    tt = np.arange(T)[:, None]
    return out_p * np.where((tt + 1) % 2 == 0, 1.0, -1.0).astype(np.float32)


if __name__ == "__main__":
    # CoreSim selftest with a short schedule (no hardware needed).
    import sys
    import time

    SG, NB, NM3, TL = 2, 5, 2, 105
    T_test = (SG + NB) * KB + TL
    nc = build_program(ser_groups=SG, n_blocks=NB, n_m3=NM3, tail=TL)
    _drop_redundant_self_waits(nc)
    print("program built, T_test =", T_test, flush=True)

    sys.path.insert(0, "/root/problem")
    import jax
    jax.config.update("jax_platform_name", "cpu")
    import reference

    inputs = reference.setup_inputs()
    inputs = {k: np.asarray(v) for k, v in inputs.items()}
    in_maps = _marshal_inputs(inputs)

    from concourse.bass_interp import MultiCoreSim

    t0 = time.time()
    sim = MultiCoreSim(nc, NCORES)
    for i in range(NCORES):
        for k, v in in_maps[i].items():
            sim.cores[i].tensor(k)[:] = v
    sim.simulate()
    print("sim time", time.time() - t0, flush=True)
    got = np.array(sim.cores[0].tensor("out"))

    w = np.load("/tmp/w_host.npy").astype(np.float32)
    s_init = np.asarray(inputs["start_part"])[-1].astype(np.float32)
    exp = _host_device_sim(w, s_init, SG, NB, NM3, TL)
    err = np.abs(got - exp)
    print("vs host-device-sim: absmax", err.max(),
          "rel", np.linalg.norm(got - exp) / max(np.linalg.norm(exp), 1e-9))
    # also vs plain serial recurrence (informative)
    sref = s_init.copy()
    ser = np.empty((T_test, N), np.float32)
    for t in range(T_test):
        sref = (np.tanh((sref @ w).astype(np.float32)) - sref).astype(np.float32)
        ser[t] = sref
    d2 = got - ser
    print("vs plain serial: absmax", np.abs(d2).max(),
          "rel", np.linalg.norm(d2) / np.linalg.norm(ser))



# revision 33
# speedup vs baseline: 2.0411x; 1.0912x over previous
"""Trainium2 Bass kernel for nn_DCGAN_G (DCGAN generator + 69-neuron spiking scan).

Strategy (8 NeuronCores, SPMD):
  A. W_in matvec (12800x2048) row-sharded 8x -> AllGather h1 (12800).
  B. DCGAN conv stack replicated on every core (tiny: ~3 GMAC).
  C. W_d2 matvec (4761x6400) row-sharded 8x -> AllGather w (69x69).
  D. 99800-step spiking recurrence in "primed" coordinates
     s'_t = (-1)^t s_t (tanh odd => u'_t = tanh(s'_t @ w)):
       serial phase (3048 steps), y'-space 2-op steps:
         y'_{t+1} = y'_t - u'_t @ w  (PSUM-accumulating matmul + tanh);
         trajectory emitted t-major per 127-step group (PE transpose +
         prefix matmul + sign multiply + direct DMA to the output).
       pipelined blocked-Picard phase: 127-step blocks, M in {2,3}
       iterations of {S' = prefix(U', s'0) via matmul with a triangular
       constant; Y' = S'@w; U' = tanh(Y')}.  Software-pipelined across
       blocks with "last-iteration-true" (LIT) semantics: iterations
       1..M-1 use a previewed start state (prefix-sum of the previous
       block's U^{(M-1)} via a ones-column matmul), only the final
       iteration waits for the previous block's converged end state.
       Critical path per block = 6 engine ops instead of 4M+2.
     Outputs are produced t-major per block (finish matmul with the
     triangular constant -> sign multiply -> batched DMA straight into
     the (T,69) output), eliminating the i-major trajectory round-trip
     and final transpose pass entirely.
     Host-validated (exact op-order mirror) vs jax ref: rel ~ 1.0e-3.
"""
import numpy as np

import bass_rust
import concourse.bass as bass
import concourse.mybir as mybir
from concourse.bass_utils import run_bass_kernel_spmd
from concourse.tile import TileContext
from concourse.vector_clock import ScopedClock

f32 = mybir.dt.float32
AF = mybir.ActivationFunctionType
OP = mybir.AluOpType
AX = mybir.AxisListType

T_FULL = 99800
N = 69
NCORES = 8
EPS = 1e-5
MROWS_A = 1600        # W_in rows per core
MROWS_C = 596         # W_d2 rows per core (8*596=4768 >= 4761)
KB = 127              # picard block length / serial group length
SER_G = 24            # serial groups (24*127 = 3048 serial steps)
N_M3 = 0              # all picard blocks at M=2 (host-validated 1.6e-3)
TAIL = 105            # tail block length (3048 + 761*127 + 105 == 99800)
DMA_GRP = 8           # picard blocks per output DMA


# ---------------------------------------------------------------------------
# walrus workaround: CTRL-type instructions accept at most 1 sem wait, but the
# TileContext tail drain gets one wait per active proc. Split across drains.
def _patched_drain_and_barrier(self, tick_clock, wait_clock):
    drain_inst = self.nc.sync.drain()
    wait_clock.add_sem_waits(
        drain_inst.ins, ScopedClock({None: tick_clock.global_clock})
    )
    si = drain_inst.ins.sync_info
    waits = list(si.on_wait) if si is not None else []
    if len(waits) > 1:
        drain_inst.ins.sync_info = bass_rust.SyncInfo(
            on_wait=waits[:1], on_update=list(si.on_update)
        )
        for i in range(1, len(waits)):
            extra = self.nc.sync.drain()
            extra.ins.sync_info = bass_rust.SyncInfo(
                on_wait=waits[i : i + 1], on_update=[]
            )
    self.nc.all_engine_barrier()
    assert self.sems is not None
    popped = self.nc._tile_sem_poison_stack.pop()
    assert popped is self._sem_poison
    self.nc.clear_and_free_semaphores(list(self.sems.allocated().values()))
    self.nc.all_engine_barrier()


TileContext._drain_and_barrier = _patched_drain_and_barrier
# ---------------------------------------------------------------------------


def _split_excess_waits(nc, max_waits=1):
    """This walrus build accepts at most one sem wait per instruction; move
    excess waits onto single-wait NOPs inserted just before the owner."""
    n_split = 0
    for f in nc.m.functions:
        for b in f.blocks:
            insts = list(b.instructions)
            out = []
            changed = False
            for inst in insts:
                si = inst.sync_info
                waits = list(si.on_wait) if si is not None else []
                if len(waits) > max_waits:
                    changed = True
                    for i, w in enumerate(waits[max_waits:]):
                        nop = mybir.InstNoOp(
                            name=f"wsp_{inst.name}_{i}", ins=[], outs=[])
                        nop.engine = inst.engine
                        nop.sync_info = bass_rust.SyncInfo(
                            on_wait=[w], on_update=[])
                        out.append(nop)
                        n_split += 1
                    inst.sync_info = bass_rust.SyncInfo(
                        on_wait=waits[:max_waits], on_update=list(si.on_update))
                out.append(inst)
            if changed:
                b.instructions = out
    return n_split


def _drop_redundant_self_waits(nc, margin=64):
    """Drop semaphore waits that are provably satisfied at issue time:
    a wait on a sem that is (a) only ever incremented by compute
    instructions of the SAME engine as the waiter, and (b) whose wait
    value is at least `margin` increments behind the number of such
    increments emitted earlier in program order.  In-order engines
    guarantee those increments completed long before the waiter issues
    (margin covers exec-queue depth + write drain).  Fewer multi-wait
    instructions means fewer wait-split NOPs on the sequencers."""
    DMA_OPS = {"DMACopy", "TriggeredCopy", "DMATranspose", "CollectiveCompute"}
    n_drop = 0
    for f in nc.m.functions:
        # pass 1: sem id -> set of (engine, is_dma) updaters
        owners = {}
        for b in f.blocks:
            for inst in b.instructions:
                si = inst.sync_info
                if si is None:
                    continue
                for u in si.on_update:
                    key = u.id
                    owners.setdefault(key, set()).add(
                        (inst.engine, inst.opcode in DMA_OPS))
        solo = {
            sid: next(iter(s))[0]
            for sid, s in owners.items()
            if len(s) == 1 and not next(iter(s))[1]
        }
        # pass 2: walk in order, count increments, drop stale self-waits
        counts = {}
        for b in f.blocks:
            for inst in b.instructions:
                si = inst.sync_info
                if si is None:
                    continue
                waits = list(si.on_wait)
                kept = []
                for w in waits:
                    sid = w.id
                    if (sid in solo and solo[sid] == inst.engine
                            and w.wait_mode == "sem-ge-imm"
                            and w.wait_value is not None
                            and w.wait_value <= counts.get(sid, 0) - margin):
                        n_drop += 1
                        continue
                    kept.append(w)
                if len(kept) != len(waits):
                    inst.sync_info = bass_rust.SyncInfo(
                        on_wait=kept, on_update=list(si.on_update))
                for u in si.on_update:
                    if u.update_mode == "sem-inc":
                        counts[u.id] = counts.get(u.id, 0) + (
                            u.update_value or 1)
    return n_drop


def _pad_w5(w5):
    """(1,64,4,4) -> (4,4,64,32) with real weights in out-column 0."""
    t = np.zeros((4, 4, 64, 32), np.float32)
    t[:, :, :, 0:1] = w5.transpose(2, 3, 1, 0)
    return np.ascontiguousarray(t)


def _col_major_pad(v, ncols):
    """(n,) -> (128, ncols) with element m at [m % 128, m // 128], zero pad."""
    out = np.zeros(128 * ncols, np.float32)
    out[: v.shape[0]] = v
    return np.ascontiguousarray(out.reshape(ncols, 128).T)


def build_program(ser_groups=SER_G, n_blocks=None, n_m3=N_M3, tail=TAIL,
                  with_scan=True, timing=False, conv_lvl=9, ser_probe=0):
    if n_blocks is None:
        n_blocks = (T_FULL - tail) // KB - ser_groups
    assert ser_groups % 2 == 0, "sign-tile parity assumes even ser_groups"
    T = (ser_groups + n_blocks) * KB + tail
    nc = bass.Bass()

    # ---- inputs ----
    x_cols = nc.declare_dram_parameter("x_cols", [128, 16], f32, isOutput=False)
    win_t = nc.declare_dram_parameter("win_t", [2048, MROWS_A], f32, isOutput=False)
    bin_c = nc.declare_dram_parameter("bin_c", [128, 13], f32, isOutput=False)
    w1t = nc.declare_dram_parameter("w1t", [4, 4, 512, 512], f32, isOutput=False)
    w2t = nc.declare_dram_parameter("w2t", [4, 4, 512, 256], f32, isOutput=False)
    w3t = nc.declare_dram_parameter("w3t", [4, 4, 256, 128], f32, isOutput=False)
    w4t = nc.declare_dram_parameter("w4t", [4, 4, 128, 64], f32, isOutput=False)
    w5t = nc.declare_dram_parameter("w5t", [4, 4, 64, 32], f32, isOutput=False)
    g_all = nc.declare_dram_parameter("g_all", [128, 8], f32, isOutput=False)
    be_all = nc.declare_dram_parameter("be_all", [128, 8], f32, isOutput=False)
    wd2_t = nc.declare_dram_parameter("wd2_t", [6400, MROWS_C], f32, isOutput=False)
    bd2_c = nc.declare_dram_parameter("bd2_c", [128, 5], f32, isOutput=False)
    s0_in = nc.declare_dram_parameter("s0", [N, 1], f32, isOutput=False)
    ident_in = nc.declare_dram_parameter("ident", [128, 128], f32, isOutput=False)
    mtri_in = nc.declare_dram_parameter("mtri", [128, 128], f32, isOutput=False)
    ones_in = nc.declare_dram_parameter("ones", [128, 128], f32, isOutput=False)
    sgne_in = nc.declare_dram_parameter("sgne", [KB, N], f32, isOutput=False)
    sgno_in = nc.declare_dram_parameter("sgno", [KB, N], f32, isOutput=False)
    if with_scan:
        out_traj = nc.declare_dram_parameter("out", [T, N], f32, isOutput=True)
    else:
        w_out = nc.declare_dram_parameter("w_out", [N, N], f32, isOutput=True)

    # ---- internal DRAM ----
    h_shard = nc.dram_tensor("h_shard", [MROWS_A], f32)
    h_full = nc.dram_tensor("h_full", [NCORES * MROWS_A], f32, addr_space="Shared")
    c_scr = nc.dram_tensor("c_scr", [32, 6400], f32)
    wd_shard = nc.dram_tensor("wd_shard", [MROWS_C], f32)
    w_full = nc.dram_tensor("w_full", [NCORES * MROWS_C], f32, addr_space="Shared")

    with TileContext(nc) as tc:
        # ================= Phase A: h = W_in @ x + b_in (sharded) ==========
        with (
            tc.tile_pool(name="a_const", bufs=1) as acp,
            tc.tile_pool(name="a_slab", bufs=2) as asp,
            tc.tile_pool(name="a_ps", bufs=1, space="PSUM") as aps,
        ):
            xc = acp.tile([128, 16], f32)
            nc.sync.dma_start(out=xc[:, :], in_=x_cols[:, :])
            bc = acp.tile([128, 13], f32)
            nc.sync.dma_start(out=bc[:, :], in_=bin_c[:, :])
            hc = acp.tile([128, 13], f32)
            for jlo, jhi in ((0, 8), (8, 13)):
                ptiles = {}
                for j in range(jlo, jhi):
                    pt = aps.tile([128, 1], f32, tag=f"hps{j - jlo}", name=f"hps{j}")
                    ptiles[j] = pt
                for k in range(16):
                    gw = min(128 * jhi, MROWS_A) - 128 * jlo
                    slab = asp.tile([128, 1024], f32, tag="aslab")
                    nc.sync.dma_start(
                        out=slab[:, :gw],
                        in_=win_t[128 * k : 128 * (k + 1),
                                  128 * jlo : 128 * jlo + gw])
                    for j in range(jlo, jhi):
                        cj = 128 if j < 12 else 64
                        jj = j - jlo
                        nc.tensor.matmul(
                            ptiles[j][:cj, :],
                            slab[:, 128 * jj : 128 * jj + cj],
                            xc[:, k : k + 1],
                            start=(k == 0),
                            stop=(k == 15),
                        )
                for j in range(jlo, jhi):
                    cj = 128 if j < 12 else 64
                    nc.vector.tensor_tensor(
                        out=hc[:cj, j : j + 1], in0=ptiles[j][:cj, :],
                        in1=bc[:cj, j : j + 1], op=OP.add)
            for j in range(13):
                cj = 128 if j < 12 else 64
                nc.sync.dma_start(
                    out=h_shard[128 * j : 128 * j + cj], in_=hc[:cj, j])
        if timing:
            # timing build (TimelineSim is single-core): local DMA stand-in;
            # the analytic collective cost is added by the caller.
            nc.sync.dma_start(out=h_full[0:MROWS_A], in_=h_shard[:])
        else:
            nc.gpsimd.collective_compute(
                "AllGather", OP.bypass, replica_groups=[list(range(NCORES))],
                ins=[h_shard[:]], outs=[h_full[:]])

        # ================= Phase B: conv stack (replicated) ================
        _lvl = conv_lvl  # conv-layer bisection gate (9 = all layers)
        h2d = h_full.rearrange("(c hw) -> c hw", hw=25)
        gsl = {1: (0, 4), 2: (4, 2), 3: (6, 1), 4: (7, 1)}  # (col offset, ncols)

        with (
            tc.tile_pool(name="bn_const", bufs=1) as bnp,
            tc.tile_pool(name="conv_ps", bufs=1, space="PSUM") as bps,
        ):
            g_sb = bnp.tile([128, 8], f32)
            nc.sync.dma_start(out=g_sb[:, :], in_=g_all[:, :])
            be_sb = bnp.tile([128, 8], f32)
            nc.sync.dma_start(out=be_sb[:, :], in_=be_all[:, :])

            def bn_relu(raw, hw, cch, lidx, j, out_ap):
                """BatchNorm(train) + ReLU from raw (cch,hw) into out_ap."""
                with tc.tile_pool(name=f"bn{lidx}_{j}", bufs=1) as p:
                    s1 = p.tile([cch, 1], f32, tag="s1")
                    nc.vector.tensor_reduce(s1[:, :], raw, axis=AX.X, op=OP.add)
                    mean = p.tile([cch, 1], f32, tag="mean")
                    nc.vector.tensor_scalar_mul(mean[:, :], s1[:, :], 1.0 / hw)
                    sq = p.tile([cch, hw], f32, tag="sq")
                    nc.vector.tensor_tensor(out=sq[:, :], in0=raw, in1=raw, op=OP.mult)
                    s2 = p.tile([cch, 1], f32, tag="s2")
                    nc.vector.tensor_reduce(s2[:, :], sq[:, :], axis=AX.X, op=OP.add)
                    ex2 = p.tile([cch, 1], f32, tag="ex2")
                    nc.vector.tensor_scalar_mul(ex2[:, :], s2[:, :], 1.0 / hw)
                    msq = p.tile([cch, 1], f32, tag="msq")
                    nc.vector.tensor_tensor(
                        out=msq[:, :], in0=mean[:, :], in1=mean[:, :], op=OP.mult)
                    var = p.tile([cch, 1], f32, tag="var")
                    nc.vector.tensor_tensor(
                        out=var[:, :], in0=ex2[:, :], in1=msq[:, :], op=OP.subtract)
                    vps = p.tile([cch, 1], f32, tag="vps")
                    nc.vector.tensor_scalar_add(vps[:, :], var[:, :], EPS)
                    sd = p.tile([cch, 1], f32, tag="sd")
                    nc.scalar.activation(sd[:, :], vps[:, :], AF.Sqrt)
                    rstd = p.tile([cch, 1], f32, tag="rstd")
                    nc.vector.reciprocal(rstd[:, :], sd[:, :])
                    co, _ = gsl[lidx]
                    scale = p.tile([cch, 1], f32, tag="scale")
                    nc.vector.tensor_tensor(
                        out=scale[:, :], in0=g_sb[:cch, co + j : co + j + 1],
                        in1=rstd[:, :], op=OP.mult)
                    t1 = p.tile([cch, 1], f32, tag="t1")
                    nc.vector.tensor_tensor(
                        out=t1[:, :], in0=mean[:, :], in1=scale[:, :], op=OP.mult)
                    bia = p.tile([cch, 1], f32, tag="bia")
                    nc.vector.tensor_tensor(
                        out=bia[:, :], in0=be_sb[:cch, co + j : co + j + 1],
                        in1=t1[:, :], op=OP.subtract)
                    nc.scalar.activation(
                        out_ap, raw, AF.Relu, bias=bia[:, :], scale=scale[:, :])

            # ---- L1: up2(h:512x5x5)->512x10x10 conv 512->512 ----
            with (
                tc.tile_pool(name="l1_in", bufs=1) as l1i,
                tc.tile_pool(name="l1_w", bufs=2) as l1w,
                tc.tile_pool(name="l1_out", bufs=1) as l1o,
            ):
                pads1 = []
                for j in range(4):
                    hm = l1i.tile([128, 25], f32, tag=f"hm{j}")
                    nc.sync.dma_start(out=hm[:, :], in_=h2d[128 * j : 128 * (j + 1), :])
                    pad = l1i.tile([128, 13 * 13], f32, tag=f"pad1_{j}")
                    nc.vector.memset(pad[:, :], 0.0)
                    pv = pad[:, :].rearrange("c (h w) -> c h w", h=13)
                    hv = hm[:, :].rearrange("c (h w) -> c h w", h=5)
                    for a in range(2):
                        for b in range(2):
                            nc.vector.tensor_copy(
                                pv[:, a + 1 : a + 11 : 2, b + 1 : b + 11 : 2], hv[:, :, :])
                    pads1.append(pad)
                ps1s = []
                for jo in range(4):
                    p1 = bps.tile([128, 100], f32, tag=f"l1ps{jo}", name=f"l1ps{jo}")
                    ps1s.append(p1)
                nmm = 0
                for ji in range(4):
                    for dy in range(4):
                        for dx in range(4):
                            slab = l1w.tile([128, 512], f32, tag="w1slab")
                            nc.sync.dma_start(
                                out=slab[:, :],
                                in_=w1t[dy, dx, 128 * ji : 128 * (ji + 1), :])
                            rhs = pads1[ji][:, :].rearrange(
                                "c (h w) -> c h w", h=13)[:, dy : dy + 10, dx : dx + 10]
                            for jo in range(4):
                                nc.tensor.matmul(
                                    ps1s[jo][:, :],
                                    slab[:, 128 * jo : 128 * (jo + 1)], rhs,
                                    start=(nmm == 0), stop=(nmm == 63))
                            nmm += 1
                pads2 = []
                for jo in range(4):
                    raw = l1o.tile([128, 100], f32, tag=f"raw1_{jo}")
                    nc.vector.tensor_copy(raw[:, :], ps1s[jo][:, :])
                    relu = l1o.tile([128, 100], f32, tag=f"relu1_{jo}")
                    bn_relu(raw[:, :], 100, 128, 1, jo, relu[:, :])
                    pad = l1o.tile([128, 23 * 23], f32, tag=f"pad2_{jo}")
                    nc.vector.memset(pad[:, :], 0.0)
                    pv = pad[:, :].rearrange("c (h w) -> c h w", h=23)
                    rv = relu[:, :].rearrange("c (h w) -> c h w", h=10)
                    for a in range(2):
                        for b in range(2):
                            nc.vector.tensor_copy(
                                pv[:, a + 1 : a + 21 : 2, b + 1 : b + 21 : 2], rv[:, :, :])
                    pads2.append(pad)

                if _lvl >= 2:
                  # ---- L2: 512x20x20 conv 512->256 ----
                  with (
                      tc.tile_pool(name="l2_w", bufs=2) as l2w,
                      tc.tile_pool(name="l2_out", bufs=1) as l2o,
                  ):
                      psA = bps.tile([128, 400], f32, tag="cpsA")
                      psB = bps.tile([128, 400], f32, tag="cpsB")
                      nmm = 0
                      for ji in range(4):
                          for dy in range(4):
                              for dx in range(4):
                                  slab = l2w.tile([128, 256], f32, tag="w2slab")
                                  nc.sync.dma_start(
                                      out=slab[:, :],
                                      in_=w2t[dy, dx, 128 * ji : 128 * (ji + 1), :])
                                  rhs = pads2[ji][:, :].rearrange(
                                      "c (h w) -> c h w", h=23)[:, dy : dy + 20, dx : dx + 20]
                                  nc.tensor.matmul(
                                      psA[:, :], slab[:, 0:128], rhs,
                                      start=(nmm == 0), stop=(nmm == 63))
                                  nc.tensor.matmul(
                                      psB[:, :], slab[:, 128:256], rhs,
                                      start=(nmm == 0), stop=(nmm == 63))
                                  nmm += 1
                      pads3 = []
                      for jo, ps in enumerate((psA, psB)):
                          raw = l2o.tile([128, 400], f32, tag=f"raw2_{jo}")
                          nc.vector.tensor_copy(raw[:, :], ps[:, :])
                          relu = l2o.tile([128, 400], f32, tag=f"relu2_{jo}")
                          bn_relu(raw[:, :], 400, 128, 2, jo, relu[:, :])
                          pad = l2o.tile([128, 43 * 43], f32, tag=f"pad3_{jo}")
                          nc.vector.memset(pad[:, :], 0.0)
                          pv = pad[:, :].rearrange("c (h w) -> c h w", h=43)
                          rv = relu[:, :].rearrange("c (h w) -> c h w", h=20)
                          for a in range(2):
                              for b in range(2):
                                  nc.vector.tensor_copy(
                                      pv[:, a + 1 : a + 41 : 2, b + 1 : b + 41 : 2],
                                      rv[:, :, :])
                          pads3.append(pad)

                      if _lvl >= 3:
                        # ---- L3: 256x40x40 conv 256->128 ----
                        with (
                            tc.tile_pool(name="l3_w", bufs=1) as l3w,
                            tc.tile_pool(name="l3_out", bufs=1) as l3o,
                        ):
                            wsl3 = l3w.tile([128, 32 * 128], f32)
                            for ji in range(2):
                                for dy in range(4):
                                    for dx in range(4):
                                        si = (ji * 16 + dy * 4 + dx) * 128
                                        nc.sync.dma_start(
                                            out=wsl3[:, si : si + 128],
                                            in_=w3t[dy, dx, 128 * ji : 128 * (ji + 1), :])
                            raw3 = l3o.tile([128, 1600], f32)
                            for st in range(4):
                                ps = bps.tile([128, 400], f32, tag="cps", bufs=2)
                                nmm = 0
                                for ji in range(2):
                                    for dy in range(4):
                                        for dx in range(4):
                                            si = (ji * 16 + dy * 4 + dx) * 128
                                            rhs = pads3[ji][:, :].rearrange(
                                                "c (h w) -> c h w", h=43)[
                                                :, st * 10 + dy : st * 10 + dy + 10,
                                                dx : dx + 40]
                                            nc.tensor.matmul(
                                                ps[:, :], wsl3[:, si : si + 128], rhs,
                                                start=(nmm == 0), stop=(nmm == 31))
                                            nmm += 1
                                nc.vector.tensor_copy(
                                    raw3[:, 400 * st : 400 * (st + 1)], ps[:, :])
                            relu3 = l3o.tile([128, 1600], f32)
                            bn_relu(raw3[:, :], 1600, 128, 3, 0, relu3[:, :])
                            pad4 = l3o.tile([128, 83 * 83], f32)
                            nc.vector.memset(pad4[:, :], 0.0)
                            pv = pad4[:, :].rearrange("c (h w) -> c h w", h=83)
                            rv = relu3[:, :].rearrange("c (h w) -> c h w", h=40)
                            for a in range(2):
                                for b in range(2):
                                    nc.vector.tensor_copy(
                                        pv[:, a + 1 : a + 81 : 2, b + 1 : b + 81 : 2],
                                        rv[:, :, :])

                            if _lvl >= 4:
                              # ---- L4: 128x80x80 conv 128->64 ----
                              with (
                                  tc.tile_pool(name="l4_w", bufs=1) as l4w,
                                  tc.tile_pool(name="l4_out", bufs=1) as l4o,
                              ):
                                  wsl4 = l4w.tile([128, 16 * 64], f32)
                                  for dy in range(4):
                                      for dx in range(4):
                                          si = (dy * 4 + dx) * 64
                                          nc.sync.dma_start(
                                              out=wsl4[:, si : si + 64],
                                              in_=w4t[dy, dx, :, :])
                                  raw4 = l4o.tile([64, 6400], f32)
                                  for st in range(16):
                                      ps = bps.tile([64, 400], f32, tag="cps", bufs=2)
                                      nmm = 0
                                      for dy in range(4):
                                          for dx in range(4):
                                              si = (dy * 4 + dx) * 64
                                              rhs = pad4[:, :].rearrange(
                                                  "c (h w) -> c h w", h=83)[
                                                  :, st * 5 + dy : st * 5 + dy + 5,
                                                  dx : dx + 80]
                                              nc.tensor.matmul(
                                                  ps[:, :], wsl4[:, si : si + 64], rhs,
                                                  start=(nmm == 0), stop=(nmm == 15))
                                              nmm += 1
                                      nc.vector.tensor_copy(
                                          raw4[:, 400 * st : 400 * (st + 1)], ps[:, :])
                                  pad5 = l4o.tile([64, 83 * 83], f32)
                                  nc.vector.memset(pad5[:, :], 0.0)
                                  pv5 = pad5[:, :].rearrange("c (h w) -> c h w", h=83)[
                                      :, 1:81, 1:81]
                                  bn_relu(raw4[:, :], 6400, 64, 4, 0, pv5)

                                  if _lvl >= 5:
                                    # ---- L5: 64x80x80 conv 64->1 + tanh -> c ----
                                    with (
                                        tc.tile_pool(name="l5_w", bufs=1) as l5w,
                                        tc.tile_pool(name="l5_out", bufs=1) as l5o,
                                    ):
                                        wsl5 = l5w.tile([64, 16 * 32], f32)
                                        for dy in range(4):
                                            for dx in range(4):
                                                _p5 = (dy * 4 + dx) * 32
                                                nc.sync.dma_start(
                                                    out=wsl5[:, _p5 : _p5 + 32],
                                                    in_=w5t[dy, dx, :, :])
                                        for st in range(16):
                                            ps = bps.tile([32, 400], f32, tag="cps", bufs=2)
                                            nmm = 0
                                            for dy in range(4):
                                                for dx in range(4):
                                                    rhs = pad5[:, :].rearrange(
                                                        "c (h w) -> c h w", h=83)[
                                                        :, st * 5 + dy : st * 5 + dy + 5,
                                                        dx : dx + 80]
                                                    _p5 = (dy * 4 + dx) * 32
                                                    nc.tensor.matmul(
                                                        ps[:, :],
                                                        wsl5[:, _p5 : _p5 + 32],
                                                        rhs,
                                                        start=(nmm == 0), stop=(nmm == 15))
                                                    nmm += 1
                                            c32 = l5o.tile([32, 400], f32, tag="c32", name=f"c32_{st}")
                                            nc.scalar.activation(c32[:, :], ps[:, :], AF.Tanh)
                                            nc.sync.dma_start(
                                                out=c_scr[:, 400 * st : 400 * (st + 1)], in_=c32[:, :])

        # ================= Phase C: w = W_d2 @ c + b_d2 (sharded) ==========
        _skip_c = False
        if not _skip_c:
          with (
              tc.tile_pool(name="c_const", bufs=1) as ccp,
              tc.tile_pool(name="c_slab", bufs=2) as csp,
              tc.tile_pool(name="c_ps", bufs=1, space="PSUM") as cps,
          ):
              c_cols = ccp.tile([128, 50], f32)
              nc.sync.dma_start(
                  out=c_cols[:, :], in_=c_scr[0, :].rearrange("(f p) -> p f", p=128))
              bdc = ccp.tile([128, 5], f32)
              nc.sync.dma_start(out=bdc[:, :], in_=bd2_c[:, :])
              wtiles = {}
              for j in range(5):
                  wt_ps = cps.tile([128, 1], f32, tag=f"wps{j}", name=f"wps{j}")
                  wtiles[j] = wt_ps
              for k in range(50):
                  slab = csp.tile([128, MROWS_C], f32, tag="cslab")
                  nc.sync.dma_start(
                      out=slab[:, :], in_=wd2_t[128 * k : 128 * (k + 1), :])
                  for j in range(5):
                      cj = 128 if j < 4 else 84
                      nc.tensor.matmul(
                          wtiles[j][:cj, :], slab[:, 128 * j : 128 * j + cj],
                          c_cols[:, k : k + 1], start=(k == 0), stop=(k == 49))
              wdc = ccp.tile([128, 5], f32)
              for j in range(5):
                  cj = 128 if j < 4 else 84
                  nc.vector.tensor_tensor(
                      out=wdc[:cj, j : j + 1], in0=wtiles[j][:cj, :],
                      in1=bdc[:cj, j : j + 1], op=OP.add)
              for j in range(5):
                  cj = 128 if j < 4 else 84
                  nc.sync.dma_start(
                      out=wd_shard[128 * j : 128 * j + cj], in_=wdc[:cj, j])
        if not _skip_c:
            if timing:
                nc.sync.dma_start(out=w_full[0:MROWS_C], in_=wd_shard[:])
            else:
                nc.gpsimd.collective_compute(
                    "AllGather", OP.bypass, replica_groups=[list(range(NCORES))],
                    ins=[wd_shard[:]], outs=[w_full[:]])

        if not with_scan:
            with tc.tile_pool(name="wout", bufs=1) as wop:
                w_sb0 = wop.tile([N, N], f32)
                nc.sync.dma_start(
                    out=w_sb0[:, :],
                    in_=w_full[0 : N * N].rearrange("(j i) -> j i", i=N))
                nc.sync.dma_start(out=w_out[:, :], in_=w_sb0[:, :])

        # ================= Phase D: spiking scan =========================
        if with_scan:
          ms = [3] * min(n_m3, n_blocks) + [2] * max(0, n_blocks - n_m3)
          kbs = [KB] * n_blocks
          if tail:
              ms.append(3)
              kbs.append(tail)
          nbt = len(ms)           # picard blocks incl tail
          ser_steps = ser_groups * KB
          with (
              tc.tile_pool(name="d_const", bufs=1) as dcp,
              tc.tile_pool(name="d_sb", bufs=2) as dsb,
          ):
            w_sb = dcp.tile([N, N], f32)
            nc.sync.dma_start(
                out=w_sb[:, :],
                in_=w_full[0 : N * N].rearrange("(j i) -> j i", i=N))
            wneg = dcp.tile([N, N], f32)
            nc.vector.tensor_scalar_mul(wneg[:, :], w_sb[:, :], -1.0)
            mtri = dcp.tile([128, 128], f32)
            nc.sync.dma_start(out=mtri[:, :], in_=mtri_in[:, :])
            ident = dcp.tile([128, 128], f32)
            nc.sync.dma_start(out=ident[:, :], in_=ident_in[:, :])
            onesm = dcp.tile([128, 128], f32)
            nc.sync.dma_start(out=onesm[:, :], in_=ones_in[:, :])
            sgne = dcp.tile([KB, N], f32)
            nc.sync.dma_start(out=sgne[:, :], in_=sgne_in[:, :])
            sgno = dcp.tile([KB, N], f32)
            nc.sync.dma_start(out=sgno[:, :], in_=sgno_in[:, :])
            s0c = dcp.tile([N, 1], f32)
            nc.sync.dma_start(out=s0c[:, :], in_=s0_in[:, :])

            # picard block state tiles (allocated up front so the serial
            # handoff can fill block 0 before the serial PSUM pool closes)
            ms = list(ms)
            ub_t = [None] * nbt
            s0_t = [None] * nbt          # [1,N] start-state rows (partition 0)
            st_t = [None] * nbt
            ub_t[0] = dsb.tile([KB, N], f32, tag="ub", bufs=3, name="ub0")
            s0_t[0] = dsb.tile([1, N], f32, tag="s0row", bufs=3, name="s0r0")

            # ---------- serial phase: 2-op steps, t-major output ----------
            # ub_ser: cols 0..126 = u' of the group, col 127 = group start s'
            serial_psum = tc.tile_pool(name="d_ps_ser", bufs=2, space="PSUM")
            dps = serial_psum.__enter__()
            ub_ser = dcp.tile([N, 128], f32)
            nc.vector.tensor_copy(ub_ser[:, 127:128], s0c[:, :])
            ybank = dps.tile([N, 1], f32, tag="ybank", name="ybank", bufs=1)
            nc.tensor.matmul(
                ybank[:, :], w_sb[:, :], s0c[:, :], start=True, stop=True)

            ser_dma_grp = 4       # serial groups per output DMA
            obufS = None
            last_ubT = None       # SBUF [128,N] of the last serial group
            last_stS = None       # PSUM [128,N] finish of the last group
            for g in range(ser_groups):
                for k in range(KB):
                    nc.scalar.activation(
                        ub_ser[:, k : k + 1], ybank[:, :], AF.Tanh)
                    nc.tensor.matmul(
                        ybank[:, :], wneg[:, :], ub_ser[:, k : k + 1],
                        start=False, stop=True, skip_group_check=True)
                if ser_probe == 1 and g < ser_groups - 1:
                    continue
                # group recon: transpose -> prefix -> sign -> batched DMA
                ubT_ps = dps.tile([128, N], f32, tag="ubTps")
                nc.tensor.transpose(ubT_ps[:, :], ub_ser[:, :], ident[:N, :N])
                ubT_sb = dsb.tile([128, N], f32, tag="ubTsb")
                nc.vector.tensor_copy(ubT_sb[:, :], ubT_ps[:, :])
                stS_ps = dps.tile([KB, N], f32, tag="stSps")
                nc.tensor.matmul(
                    stS_ps[:, :], mtri[:, 1 : KB + 1], ubT_sb[:, :],
                    start=True, stop=True)
                gi = g % ser_dma_grp
                if gi == 0 or obufS is None:
                    obufS = dsb.tile([KB, ser_dma_grp * N], f32, tag="obufS")
                sg = sgne if g % 2 == 0 else sgno
                nc.vector.tensor_tensor(
                    out=obufS[:, gi * N : (gi + 1) * N],
                    in0=stS_ps[:, :], in1=sg[:, :], op=OP.mult)
                ngrp = min(ser_dma_grp, ser_groups - (g - gi))
                if gi == ngrp - 1:
                    r0 = (g - gi) * KB
                    nc.sync.dma_start(
                        out=out_traj[r0 : r0 + ngrp * KB, :].rearrange(
                            "(b p) n -> p b n", b=ngrp),
                        in_=obufS[:, : ngrp * N].rearrange(
                            "p (b n) -> p b n", b=ngrp))
                # next start column via single-col prefix matmul
                scol_ps = dps.tile([N, 1], f32, tag="scolps")
                nc.tensor.matmul(
                    scol_ps[:, :], ubT_sb[:, :], mtri[:, 127:128],
                    start=True, stop=True)
                if g < ser_groups - 1:
                    nc.vector.tensor_copy(ub_ser[:, 127:128], scol_ps[:, :])
                else:
                    last_ubT = ubT_sb
                    last_scol = scol_ps

            # exact serial -> block-0 handoff (inside the serial PSUM
            # scope: last_scol is a PSUM tile). The end-state column is
            # transposed into a partition-0 row.
            nc.gpsimd.tensor_copy(ub_t[0][:, :], last_ubT[0:KB, :])
            scol_sb = dsb.tile([N, 1], f32, tag="scolsb")
            nc.vector.tensor_copy(scol_sb[:, :], last_scol[:, :])
            s0T_ps = dps.tile([1, N], f32, tag="s0Tps", bufs=1)
            nc.tensor.transpose(s0T_ps[:, :], scol_sb[:, :], ident[:N, :N])
            nc.vector.tensor_copy(s0_t[0][:, :], s0T_ps[:, :])
            serial_psum.__exit__(None, None, None)

            # ---------- pipelined blocked-Picard phase (LIT) ----------
            # Per block b: seeds + preview start from block b-1's U^{(M-1)}
            # (sum via e127 matmul); only the last iteration waits for the
            # true (converged) start state of the block.
            picard_psum = tc.tile_pool(name="d_ps_pic", bufs=2, space="PSUM")
            dps = picard_psum.__enter__()
            obuf = None
            obuf_base = 0

            pend_sp = [None] * nbt    # hoisted u-part prefix (final iter)

            def emit_mm1a(b):
                """u-part prefix matmul of block b's FINAL iteration, hoisted
                off the critical chain (only needs tanh(b, M-1))."""
                kb = kbs[b]
                sp_ps = dps.tile([N, 128], f32, tag="spps")
                nc.tensor.matmul(
                    sp_ps[:, :kb], ub_t[b][:, :], mtri[0:KB, :kb],
                    start=True, stop=False, skip_group_check=True)
                pend_sp[b] = sp_ps

            def emit_iter(b, final=False):
                """One Picard iteration of block b (mm1 pair,copy,mm2,tanh).
                S' cols = u-row prefix (strict-lower mtri) + s0 broadcast."""
                kb = kbs[b]
                if final:
                    sp_ps = pend_sp[b]
                else:
                    sp_ps = dps.tile([N, 128], f32, tag="spps")
                    nc.tensor.matmul(
                        sp_ps[:, :kb], ub_t[b][:, :], mtri[0:KB, :kb],
                        start=True, stop=False, skip_group_check=True)
                nc.tensor.matmul(
                    sp_ps[:, :kb], s0_t[b][:, :], onesm[0:1, :kb],
                    start=False, stop=True, skip_group_check=True)
                sp_sb = dsb.tile([N, KB], f32, tag="spsb")
                nc.vector.tensor_copy(sp_sb[:, :kb], sp_ps[:, :kb])
                y_ps = dps.tile([KB, N], f32, tag="yps")
                nc.tensor.matmul(
                    y_ps[:kb, :], sp_sb[:, :kb], w_sb[:, :],
                    start=True, stop=True)
                nc.scalar.activation(ub_t[b][0:kb, :], y_ps[:kb, :], AF.Tanh)

            for s in range(nbt + 1):
                # --- pre-ops of block s (reads block s-1's U^{(M-1)}) ---
                if 0 < s < nbt:
                    ub_t[s] = dsb.tile([KB, N], f32, tag="ub", bufs=3,
                                       name=f"ub{s}")
                    s0_t[s] = dsb.tile([1, N], f32, tag="s0row", bufs=3,
                                       name=f"s0r{s}")
                    if True:
                        sum_ps = dps.tile([1, N], f32, tag="sumps")
                        nc.tensor.matmul(
                            sum_ps[:, :], onesm[0:KB, 0:1], ub_t[s - 1][:, :],
                            start=True, stop=True)
                        nc.vector.tensor_tensor(
                            out=s0_t[s][:, :], in0=s0_t[s - 1][:, :],
                            in1=sum_ps[:, :], op=OP.subtract)
                        nc.gpsimd.tensor_copy(
                            ub_t[s][:, :], ub_t[s - 1][:, :])
                # --- final iteration of block s-1 (u-part hoisted) ---
                if s > 0:
                    emit_iter(s - 1, final=True)
                # --- iterations 1..M-1 of block s (preview start) ---
                if s < nbt:
                    for m in range(1, ms[s]):
                        emit_iter(s)
                # --- true-start handoff for block s (after tanh(s-1, M),
                #     before the final iteration emitted next stage) ---
                if 0 < s < nbt:
                    sum2_ps = dps.tile([1, N], f32, tag="sumps",
                                       name=f"sum2_{s}")
                    nc.tensor.matmul(
                        sum2_ps[:, :], onesm[0:KB, 0:1], ub_t[s - 1][:, :],
                        start=True, stop=True)
                    nc.vector.tensor_tensor(
                        out=s0_t[s][:, :], in0=s0_t[s - 1][:, :],
                        in1=sum2_ps[:, :], op=OP.subtract)
                # --- hoist the final iteration's u-part prefix (after T1 so
                #     it cannot head-of-line block the true-start sum) ---
                if s < nbt:
                    emit_mm1a(s)
                # --- finish + output of block s-1 ---
                if s > 0:
                    b = s - 1
                    kb = kbs[b]
                    st_t[b] = dps.tile([KB, N], f32, tag="stps",
                                       name=f"st{b}")
                    nc.tensor.matmul(
                        st_t[b][:kb, :], mtri[0:KB, 1 : kb + 1], ub_t[b][:, :],
                        start=True, stop=False)
                    nc.tensor.matmul(
                        st_t[b][:kb, :], onesm[0:1, 0:kb], s0_t[b][:, :],
                        start=False, stop=True)
                    if b < n_blocks:
                        gi = b % DMA_GRP
                        if gi == 0:
                            obuf = dsb.tile([KB, DMA_GRP * N], f32,
                                            tag="obuf")
                            obuf_base = b
                        sg = sgne if b % 2 == 0 else sgno
                        nc.vector.tensor_tensor(
                            out=obuf[:, gi * N : (gi + 1) * N],
                            in0=st_t[b][:, :], in1=sg[:, :],
                            op=OP.mult)
                        ngrp = min(DMA_GRP, n_blocks - obuf_base)
                        if gi == ngrp - 1:
                            r0 = ser_steps + obuf_base * KB
                            nc.sync.dma_start(
                                out=out_traj[
                                    r0 : r0 + ngrp * KB, :].rearrange(
                                    "(b p) n -> p b n", b=ngrp),
                                in_=obuf[:, : ngrp * N].rearrange(
                                    "p (b n) -> p b n", b=ngrp))
                    else:
                        # tail block: sign pattern continues the parity of
                        # block index b (tail rows start at an odd offset)
                        sg = sgne if b % 2 == 0 else sgno
                        otail = dsb.tile([KB, N], f32, tag="otail")
                        nc.vector.tensor_tensor(
                            out=otail[:kb, :], in0=st_t[b][:kb, :],
                            in1=sg[:kb, :], op=OP.mult)
                        nc.sync.dma_start(
                            out=out_traj[T - kb : T, :], in_=otail[:kb, :])
            picard_psum.__exit__(None, None, None)

    return nc


def _marshal_inputs(inputs):
    """Build the 8 per-core input maps from the full problem inputs."""
    x = np.asarray(inputs["x"], np.float32).reshape(2048)
    win = np.asarray(inputs["W_in"], np.float32)
    b_in = np.asarray(inputs["b_in"], np.float32)
    wd2 = np.asarray(inputs["W_d2"], np.float32)
    bd2 = np.asarray(inputs["b_d2"], np.float32)
    sp = np.asarray(inputs["start_part"], np.float32)

    x_cols = np.ascontiguousarray(x.reshape(16, 128).T)
    g_all = np.zeros((128, 8), np.float32)
    be_all = np.zeros((128, 8), np.float32)
    g_all[:, 0:4] = _col_major_pad(np.asarray(inputs["g1"], np.float32), 4)
    g_all[:, 4:6] = _col_major_pad(np.asarray(inputs["g2"], np.float32), 2)
    g_all[:, 6:7] = _col_major_pad(np.asarray(inputs["g3"], np.float32), 1)
    g_all[:, 7:8] = _col_major_pad(np.asarray(inputs["g4"], np.float32), 1)
    be_all[:, 0:4] = _col_major_pad(np.asarray(inputs["be1"], np.float32), 4)
    be_all[:, 4:6] = _col_major_pad(np.asarray(inputs["be2"], np.float32), 2)
    be_all[:, 6:7] = _col_major_pad(np.asarray(inputs["be3"], np.float32), 1)
    be_all[:, 7:8] = _col_major_pad(np.asarray(inputs["be4"], np.float32), 1)
    wts = {
        "w1t": np.ascontiguousarray(
            np.asarray(inputs["w1"], np.float32).transpose(2, 3, 1, 0)),
        "w2t": np.ascontiguousarray(
            np.asarray(inputs["w2"], np.float32).transpose(2, 3, 1, 0)),
        "w3t": np.ascontiguousarray(
            np.asarray(inputs["w3"], np.float32).transpose(2, 3, 1, 0)),
        "w4t": np.ascontiguousarray(
            np.asarray(inputs["w4"], np.float32).transpose(2, 3, 1, 0)),
        "w5t": _pad_w5(np.asarray(inputs["w5"], np.float32)),
    }
    s0 = np.ascontiguousarray(sp[-1].reshape(N, 1))
    ident = np.eye(128, dtype=np.float32)
    # prefix matrix: S'[i,t] = sum_k ubT[k,i]*mtri[k,t]; strict-lower -1s
    # for the u' rows, +1 base row (127) for the s'0 term.
    mtri = np.zeros((128, 128), np.float32)
    for k in range(127):
        mtri[k, k + 1 :] = -1.0
    mtri[127, :] = 1.0
    # all-ones helper (column sums / base-row broadcasts via matmul)
    onesm = np.ones((128, 128), np.float32)
    # unpriming signs by output row parity: out[t] = (-1)^(t+1) s'_{t+1};
    # within a group starting at even global t, row j gets (-1)^(j+1).
    sgne = np.tile(
        np.where(np.arange(KB) % 2 == 0, -1.0, 1.0
                 ).astype(np.float32)[:, None], (1, N))
    sgno = -sgne

    wd2_pad = np.zeros((NCORES * MROWS_C, 6400), np.float32)
    wd2_pad[: wd2.shape[0]] = wd2
    bd2_pad = np.zeros(NCORES * MROWS_C, np.float32)
    bd2_pad[: bd2.shape[0]] = bd2

    in_maps = []
    for c in range(NCORES):
        m = {
            "x_cols": x_cols,
            "win_t": np.ascontiguousarray(
                win[MROWS_A * c : MROWS_A * (c + 1)].T),
            "bin_c": _col_major_pad(b_in[MROWS_A * c : MROWS_A * (c + 1)], 13),
            "g_all": g_all,
            "be_all": be_all,
            "wd2_t": np.ascontiguousarray(
                wd2_pad[MROWS_C * c : MROWS_C * (c + 1)].T),
            "bd2_c": _col_major_pad(bd2_pad[MROWS_C * c : MROWS_C * (c + 1)], 5),
            "s0": s0,
            "ident": ident,
            "mtri": mtri,
            "ones": onesm,
            "sgne": sgne,
            "sgno": sgno,
        }
        m.update(wts)
        in_maps.append(m)
    return in_maps


LAST_EXEC_NS = None


def kernel(**inputs) -> np.ndarray:
    global LAST_EXEC_NS
    import os

    trace = bool(os.environ.get("KERNEL_TRACE"))
    nc = build_program()
    _drop_redundant_self_waits(nc)
    _split_excess_waits(nc)
    in_maps = _marshal_inputs(inputs)
    res = run_bass_kernel_spmd(nc, in_maps, list(range(NCORES)), trace=trace)
    if res.exec_time_ns is not None:
        LAST_EXEC_NS = res.exec_time_ns
    out = np.asarray(res.results[0]["out"], np.float32)
    return out.reshape(1, T_FULL, N)


def _host_device_sim(w, s_init, ser_groups=SER_G, n_blocks=None, n_m3=N_M3,
                     tail=TAIL):
    """Numpy mirror of the device schedule (pipelined LIT semantics)."""
    if n_blocks is None:
        n_blocks = (T_FULL - tail) // KB - ser_groups
    ser_steps = ser_groups * KB
    T = ser_steps + n_blocks * KB + tail
    ms = [3] * min(n_m3, n_blocks) + [2] * max(0, n_blocks - n_m3)
    kbs = [KB] * n_blocks
    if tail:
        ms.append(3)
        kbs.append(tail)
    out_p = np.empty((T, N), np.float32)
    yp = (s_init @ w).astype(np.float32)
    sp = s_init.copy()
    ubh = np.zeros((KB, N), np.float32)
    for t in range(ser_steps):
        up = np.tanh(yp).astype(np.float32)
        ubh[t % KB] = up
        yp = (yp - (up @ w).astype(np.float32)).astype(np.float32)
        sp = (sp - up).astype(np.float32)
        out_p[t] = sp

    def prefix(s0, U, Kb):
        S = np.empty((Kb + 1, N), np.float32)
        S[0] = s0
        S[1:] = s0 - np.cumsum(U[:Kb], axis=0, dtype=np.float32)
        return S

    true_prev = out_p[ser_steps - 1].copy()   # true start of block 0 (exact)
    seeds = ubh.copy()                        # U^{(M-1)} of "block -1"
    t = ser_steps
    for b in range(len(ms)):
        M, kb = ms[b], kbs[b]
        if b == 0:
            start_all = true_prev             # exact for every iteration
            true_b = true_prev
        else:
            preview = (true_prev - seeds.sum(axis=0,
                                             dtype=np.float32)).astype(
                np.float32)
            start_all = preview
            true_b = None                     # filled after U^{(M)} known
        U = seeds.copy()
        for m in range(1, M):                 # preview iterations
            S = prefix(start_all, U, kb)
            Y = (S[:kb] @ w).astype(np.float32)
            U = np.tanh(Y).astype(np.float32)
        if b == 0:
            true_b = true_prev
        else:
            # sum2 over prev block's converged U^{(M)}
            true_b = (true_prev - prev_conv.sum(axis=0,
                                                dtype=np.float32)).astype(
                np.float32)
        seeds_next = U.copy()                 # U^{(M-1)} of this block
        S = prefix(true_b, U, kb)             # final iteration (true start)
        Y = (S[:kb] @ w).astype(np.float32)
        U = np.tanh(Y).astype(np.float32)
        St = prefix(true_b, U, kb)            # finish
        out_p[t : t + kb] = St[1 : kb + 1]
        prev_conv = U
        seeds = seeds_next
        true_prev = true_b
        t += kb
    tt = np.arange(T)[:, None]
    return out_p * np.where((tt + 1) % 2 == 0, 1.0, -1.0).astype(np.float32)


if __name__ == "__main__":
    # CoreSim selftest with a short schedule (no hardware needed).
    import sys
    import time

    SG, NB, NM3, TL = 2, 5, 2, 105
    T_test = (SG + NB) * KB + TL
    nc = build_program(ser_groups=SG, n_blocks=NB, n_m3=NM3, tail=TL)
    _drop_redundant_self_waits(nc)
    print("program built, T_test =", T_test, flush=True)

    sys.path.insert(0, "/root/problem")
    import jax
    jax.config.update("jax_platform_name", "cpu")
    import reference

    inputs = reference.setup_inputs()
    inputs = {k: np.asarray(v) for k, v in inputs.items()}
    in_maps = _marshal_inputs(inputs)

    from concourse.bass_interp import MultiCoreSim

    t0 = time.time()
    sim = MultiCoreSim(nc, NCORES)
    for i in range(NCORES):
        for k, v in in_maps[i].items():
            sim.cores[i].tensor(k)[:] = v
    sim.simulate()
    print("sim time", time.time() - t0, flush=True)
    got = np.array(sim.cores[0].tensor("out"))

    w = np.load("/tmp/w_host.npy").astype(np.float32)
    s_init = np.asarray(inputs["start_part"])[-1].astype(np.float32)
    exp = _host_device_sim(w, s_init, SG, NB, NM3, TL)
    err = np.abs(got - exp)
    print("vs host-device-sim: absmax", err.max(),
          "rel", np.linalg.norm(got - exp) / max(np.linalg.norm(exp), 1e-9))
    # also vs plain serial recurrence (informative)
    sref = s_init.copy()
    ser = np.empty((T_test, N), np.float32)
    for t in range(T_test):
        sref = (np.tanh((sref @ w).astype(np.float32)) - sref).astype(np.float32)
        ser[t] = sref
    d2 = got - ser
    print("vs plain serial: absmax", np.abs(d2).max(),
          "rel", np.linalg.norm(d2) / np.linalg.norm(ser))

